# revision 1
# baseline (speedup 1.0000x reference)
"""Distributed Trainium2 Bass kernel for the spherical-harmonic AMSE loss.

Algorithm (8 NeuronCores, m-sharded; m = 8k + core_id interleave):
  host:    longitude fold — F_m = sum_{n<360} (x[n] + (-1)^m x[n+360]) w^{mn},
           and every m on core cid has parity cid%2, so each core gets
           xT[360, (t, bc, j-pad)] bf16 folded with its own sign (halves the
           xT DMA and the stage-1 matmul work).  Per-core DFT slices
           (128-col padded for FWL) and Legendre tables legw[m][j', jt, l-pad]
           with quadrature w and the m=0 PSD halving (1/sqrt2) folded in.
  stage 1: F[m2, rows] = dftT.T @ xT            (PE, PSUM accum over lon)
           xT chunks alternate sync/scalar HW DMA queues; legw streams on
           the gpsimd SW-DGE queue from t=0.
  xbar:    F -> FT[j', (t, bc, jt, m2)]          (DMA crossbar transpose,
           per-block as soon as its 3 chunks land, on the scalar queue)
  stage 2: C[l', (lt, m, t, bc, ri)] = legw.T @ FT   (PE, PSUM accum over
           j-tiles)
  stage 3: |C|^2 and conj(P)*T products + reductions over local m (DVE),
           chunked by m-groups so it overlaps stage 2.
  AllGather [128, 192] f32 partials + local DVE tree-sum (cheaper than the
  cc-firmware AllReduce); final loss math redundantly per core.
"""
import os
import numpy as np
import ml_dtypes

os.environ.setdefault("NEURON_RT_DBG_RDH_CC", "0")

NLON = 720
NLONF = 360          # folded longitude
L = 361
EPS = 1e-7
NCORES = 8
MSLOT = 46           # m slots per core (m = 8k + core_id; zero-padded if > 360)
M2 = 2 * MSLOT       # 92 live re/im columns
M2P = 128            # padded stationary width: FWL needs exactly 128 cols
JP = 384             # padded latitude rows per (t, bc)  (3 * 128)
T = 2
BC = 16
ROWS = T * BC * JP   # 12288
CHUNK = 512
NCHUNK = ROWS // CHUNK
KT = 3               # folded: 360 = 3 * 120
KTW = 120
LP = 384             # padded l (3 * 128)
LT = 3

bf16 = ml_dtypes.bfloat16
f8 = ml_dtypes.float8_e4m3
LSC = 256.0              # fp8 legw pre-scale (keeps values in normal range)
SC = LSC * LSC           # pp/cr/ci come out scaled by LSC^2; wvec unwinds it

_CACHE = {}


def _build_tables(leg, w, weights):
    legf = np.asarray(leg, np.float32)          # [L, M, J]
    wf = np.asarray(w, np.float32)              # [J]
    legT = legf.transpose(1, 2, 0) * wf[None, :, None]   # [M, J, L]
    legT[0] *= np.float32(2.0 ** -0.5)          # uniform p = 2*sum|C|^2
    legT *= np.float32(2.0 ** 0.5)              # bake the psd 2x
    legT *= np.float32(LSC)                     # fp8 normal-range pre-scale
    legp = np.zeros((MSLOT * NCORES, JP, LP), np.float32)
    legp[:L, :L, :L] = legT
    legp = legp.reshape(MSLOT, NCORES, JP, LP).transpose(1, 0, 2, 3)
    legw = np.ascontiguousarray(
        legp.reshape(NCORES, MSLOT, 3, 128, LP).transpose(0, 1, 3, 2, 4)
    ).astype(f8)                                # [8][46, 128(j'), 3(jt), 384(l)]

    n = np.arange(NLONF, dtype=np.float64)
    m_all = np.arange(MSLOT * NCORES, dtype=np.float64)
    ang = 2.0 * np.pi * np.outer(n, m_all) / NLON
    scale = 2.0 * np.pi / NLON
    dft = np.zeros((NLONF, MSLOT * NCORES, 2), np.float64)
    dft[:, :, 0] = np.cos(ang) * scale
    dft[:, :, 1] = -np.sin(ang) * scale
    dft[:, L:, :] = 0.0
    dft = dft.reshape(NLONF, MSLOT, NCORES, 2).transpose(2, 0, 1, 3)  # [8,360,46,2]
    dftp = np.zeros((NCORES, NLONF, M2P), np.float64)
    dftp[:, :, :M2] = dft.reshape(NCORES, NLONF, M2)
    dftp = dftp.reshape(NCORES, KT, KTW, M2P).transpose(0, 2, 1, 3)  # [8,120,3,128]
    dftc = np.ascontiguousarray(dftp).astype(bf16)

    wvec = (np.tile(np.asarray(weights, np.float32), T) / (360.0 * 16.0 * SC)).reshape(16, 1)
    ones16 = np.ones((16, 1), np.float32)
    lmask = np.zeros((128, LT), np.float32)
    for lt in range(LT):
        for p in range(128):
            if lt * 128 + p < L - 1:
                lmask[p, lt] = 1.0
    return legw, dftc, wvec, ones16, lmask


def _pack_inputs(prediction, target):
    x = np.zeros((T, BC, JP, NLON), np.float32)
    x[0, :, :L] = np.asarray(prediction, np.float32).reshape(BC, L, NLON)
    x[1, :, :L] = np.asarray(target, np.float32).reshape(BC, L, NLON)
    lo, hi = x[..., :NLONF], x[..., NLONF:]
    xTE = np.ascontiguousarray(
        (lo + hi).transpose(3, 0, 1, 2).reshape(NLONF, ROWS)).astype(f8)
    xTO = np.ascontiguousarray(
        (lo - hi).transpose(3, 0, 1, 2).reshape(NLONF, ROWS)).astype(f8)
    return xTE, xTO


def _build_graph():
    import concourse.bacc as bacc
    import concourse.mybir as mybir
    from concourse.tile import TileContext

    fp32 = mybir.dt.float32
    bft = mybir.dt.bfloat16
    f8t = mybir.dt.float8e4

    nc = bacc.Bacc(None, target_bir_lowering=False)

    xT_e = nc.declare_dram_parameter("xT", [NLONF, ROWS], f8t, isOutput=False)
    legw_e = nc.declare_dram_parameter("legw", [MSLOT, 128, 3, LP], f8t, isOutput=False)
    dft_e = nc.declare_dram_parameter("dftT", [KTW, KT, M2P], bft, isOutput=False)
    wvec_e = nc.declare_dram_parameter("wvec", [16, 1], fp32, isOutput=False)
    ones_e = nc.declare_dram_parameter("ones16", [16, 1], fp32, isOutput=False)
    mask_e = nc.declare_dram_parameter("lmask", [128, LT], fp32, isOutput=False)
    out_e = nc.declare_dram_parameter("out", [1, 1], fp32, isOutput=True)

    ar_in = nc.dram_tensor("ar_in", [128, 192], fp32)
    ar_out = nc.dram_tensor("ar_out", [NCORES * 128, 192], fp32, addr_space="Shared")
    ar_in2 = nc.dram_tensor("ar_in2", [128, 128], fp32)
    ar_out2 = nc.dram_tensor("ar_out2", [NCORES * 128, 128], fp32, addr_space="Shared")

    add = mybir.AluOpType.add
    sub = mybir.AluOpType.subtract
    mult = mybir.AluOpType.mult
    amax = mybir.AluOpType.max
    amin = mybir.AluOpType.min
    bypass = mybir.AluOpType.bypass
    AF = mybir.ActivationFunctionType
    AX = mybir.AxisListType

    with TileContext(nc) as tc:
        with (
            tc.tile_pool(name="consts", bufs=1) as consts,
            tc.tile_pool(name="xp", bufs=8) as xp,
            tc.tile_pool(name="fps", bufs=2, space="PSUM") as fps,
            tc.tile_pool(name="big", bufs=1) as big,
            tc.tile_pool(name="lw", bufs=32) as lwp,
            tc.tile_pool(name="cps", bufs=3, space="PSUM") as cps,
            tc.tile_pool(name="fin", bufs=1) as fin,
        ):
            dft_sb = consts.tile([KTW, KT, M2P], bft)
            nc.sync.dma_start(dft_sb[:], dft_e[:])
            wvec_sb = consts.tile([16, 1], fp32)
            nc.sync.dma_start(wvec_sb[:], wvec_e[:])
            ones_sb = consts.tile([16, 1], fp32)
            nc.sync.dma_start(ones_sb[:], ones_e[:])
            mask_sb = consts.tile([128, LT], fp32)
            nc.sync.dma_start(mask_sb[:], mask_e[:])

            # ---- legw streams on the gpsimd SW-DGE queue from t=0; the
            # 16-buf ring holds two groups, so groups prefetch 2 ahead ----
            MGROUPS = [(0, 8), (8, 16), (16, 24), (24, 32), (32, 38), (38, 43), (43, 46)]
            lw_tiles = {}

            def load_lw(k, eng):
                # m < 24 rides gpsimd early; the rest ride the sync ring so
                # the gpsimd queue is drained when the collectives trigger
                lt0 = min(2, k // 16)   # l-tiles below the diagonal are zero
                lw = lwp.tile([128, 3, LP], f8t, name="lw")
                eng.dma_start(lw[:, :, lt0 * 128:LP],
                              legw_e[k][:, :, lt0 * 128:LP])
                lw_tiles[k] = lw

            for k in range(24):
                load_lw(k, nc.gpsimd)

            # ---- stage 1: DFT (dead dft cols zero the F pad rows) ----
            F_sb = big.tile([M2P, ROWS], bft)
            FT_sb = big.tile([128, 96 * M2P], bft)
            FT_v3 = FT_sb[:].rearrange("p (c m) -> p c m", m=M2P)
            TCH = 8
            tw = ROWS // TCH            # 1536 = 3 chunks per transpose block
            xT_v = xT_e[:].rearrange("(k p) r -> p k r", p=KTW)
            tposed = 0
            for c0 in range(0, NCHUNK, 2):
                xts, pss = [], []
                for i, c in enumerate((c0, c0 + 1)):
                    xt = xp.tile([KTW, KT, CHUNK], f8t)
                    eng = nc.sync if i == 0 else nc.scalar
                    eng.dma_start(xt[:], xT_v[:, :, c * CHUNK:(c + 1) * CHUNK])
                    xts.append(xt)
                    ps = fps.tile([M2P, CHUNK], fp32, tag="s1ps", bufs=3)
                    pss.append(ps)
                # interleave two accumulation chains: PE always has an
                # independent matmul in flight to hide the drain
                for kt in range(KT):
                    for i in (0, 1):
                        nc.tensor.matmul(
                            pss[i][:], dft_sb[:, kt, :], xts[i][:, kt, :],
                            start=(kt == 0), stop=(kt == KT - 1),
                        )
                for i in (0, 1):
                    c = c0 + i
                    nc.vector.tensor_copy(
                        F_sb[:, c * CHUNK:(c + 1) * CHUNK], pss[i][:])
                # transpose each block as soon as its 3 chunks land,
                # alternating queues to split the brief head-of-line waits
                while (tposed + 1) * 3 <= c0 + 2:
                    t = tposed
                    eng = nc.sync if t % 2 == 0 else nc.scalar
                    eng.dma_start_transpose(
                        FT_v3[:, t * (tw // 128):(t + 1) * (tw // 128), :],
                        F_sb[:, t * tw:(t + 1) * tw],
                    )
                    tposed += 1
            FT_v = FT_sb[:].rearrange(
                "p (t bc jt m) -> p t bc jt m", t=T, bc=BC, jt=3, m=M2P
            )

            # ---- stage 2: per-group C tiles [p, lts, g, t, bc, ri] so the
            # PSUM->SBUF copies stay contiguous; stage 3 trims to live lt ----
            # partial sums accumulate straight into the all-reduce payload:
            # [0:96] pp (lt,t,bc) | [96:144] cr (lt,bc) | [144:192] ci (lt,bc)
            ar_sb = fin.tile([128, 192], fp32)
            ar_pp = ar_sb[:, 0:96].rearrange("p (lt t bc) -> p lt t bc", lt=LT, t=T)
            ar_cr = ar_sb[:, 96:144].rearrange("p (lt bc) -> p lt bc", lt=LT)
            ar_ci = ar_sb[:, 144:192].rearrange("p (lt bc) -> p lt bc", lt=LT)
            # groups >= 3 accumulate into a second payload (lt1/lt2 regions
            # only) so the first all-gather can launch after group 2
            ar2_sb = fin.tile([128, 128], fp32)
            a2_pp = ar2_sb[:, 0:64].rearrange("p (lt t bc) -> p lt t bc", lt=2, t=T)
            a2_cr = ar2_sb[:, 64:96].rearrange("p (lt bc) -> p lt bc", lt=2)
            a2_ci = ar2_sb[:, 96:128].rearrange("p (lt bc) -> p lt bc", lt=2)
            # groups are aligned to the 16-m diagonal steps, so each group's
            # live l-tiles are exactly lt >= ltm with the full m-span
            LTMIN = [0, 0, 1, 1, 2, 2, 2]
            for gi, (m0, m1) in enumerate(MGROUPS):
                if gi + 3 < len(MGROUPS):
                    for k in range(*MGROUPS[gi + 3]):
                        load_lw(k, nc.sync)
                if gi == 2:
                    for k in range(*MGROUPS[6]):
                        load_lw(k, nc.sync)
                gsz = m1 - m0
                ltm = LTMIN[gi]
                nlt = LT - ltm
                Cg = big.tile([128, LT * 8 * 64], bft, name="Cg", tag="Cg", bufs=3)
                Cv = Cg[:, :nlt * gsz * 64].rearrange(
                    "p (lts g t bc ri) -> p lts g t bc ri", lts=nlt, g=gsz, t=T, bc=BC)
                for li, lt in enumerate(range(ltm, LT)):
                    ps = cps.tile([128, 512], fp32)
                    for mi in range(gsz):
                        k = m0 + mi
                        rhs = FT_v[:, :, :, :, 2 * k:2 * k + 2]
                        for jt in range(3):
                            nc.tensor.matmul(
                                ps[:, mi * 64:(mi + 1) * 64],
                                lw_tiles[k][:, jt, lt * 128:(lt + 1) * 128],
                                rhs[:, :, :, jt, :],
                                start=(jt == 0), stop=(jt == 2),
                            )
                    nc.scalar.activation(
                        Cg[:, li * gsz * 64:(li + 1) * gsz * 64],
                        ps[:, 0:gsz * 64], AF.Copy)
                # ---- stage 3 for this m-group (overlaps next group's matmuls) ----
                if gi <= 2:
                    tpp, tcr, tci, off, init = ar_pp, ar_cr, ar_ci, ltm, gi == 0
                else:
                    tpp, tcr, tci, off, init = a2_pp, a2_cr, a2_ci, ltm - 1, gi == 3
                cP = Cv[:, :, :, 0, :, :]     # [p, lts, g, bc, ri]
                cT = Cv[:, :, :, 1, :, :]
                sqg = fin.tile([128, LT * 8 * 64], bft, tag="sqg", bufs=2)
                sq_v = sqg[:, :nlt * gsz * 64].rearrange(
                    "p (lts g t bc ri) -> p lts g t bc ri", lts=nlt, g=gsz, t=T, bc=BC)
                nc.vector.tensor_tensor(sq_v, Cv, Cv, mult)
                sq_r = sqg[:, :nlt * gsz * 64].rearrange(
                    "p (lts g t bc ri) -> p lts t bc g ri", lts=nlt, g=gsz, t=T, bc=BC)
                if init:
                    nc.vector.tensor_reduce(tpp[:, off:], sq_r, axis=AX.XY, op=add)
                else:
                    rtmp = fin.tile([128, 96], fp32, tag="rtmp", bufs=2)
                    rt = rtmp[:, :nlt * 32].rearrange(
                        "p (lts t bc) -> p lts t bc", lts=nlt, t=T)
                    nc.vector.tensor_reduce(rt, sq_r, axis=AX.XY, op=add)
                    nc.vector.tensor_tensor(
                        tpp[:, off:], tpp[:, off:], rt, add)
                crg = fin.tile([128, LT * 8 * 32], bft, tag="crg", bufs=2)
                cr_v = crg[:, :nlt * gsz * 32].rearrange(
                    "p (lts g bc ri) -> p lts g bc ri", lts=nlt, g=gsz, bc=BC)
                nc.vector.tensor_tensor(cr_v, cP, cT, mult)
                cr_r = crg[:, :nlt * gsz * 32].rearrange(
                    "p (lts g bc ri) -> p lts bc g ri", lts=nlt, g=gsz, bc=BC)
                if init:
                    nc.vector.tensor_reduce(tcr[:, off:], cr_r, axis=AX.XY, op=add)
                else:
                    ctmp = fin.tile([128, 48], fp32, tag="ctmp", bufs=2)
                    ct = ctmp[:, :nlt * 16].rearrange("p (lts bc) -> p lts bc", lts=nlt)
                    nc.vector.tensor_reduce(ct, cr_r, axis=AX.XY, op=add)
                    nc.vector.tensor_tensor(
                        tcr[:, off:], tcr[:, off:], ct, add)
                cig = fin.tile([128, 2 * LT * 8 * 16], bft, tag="cig", bufs=2)
                ci_v = cig[:, :2 * nlt * gsz * 16].rearrange(
                    "p (s lts g bc) -> p s lts g bc", s=2, lts=nlt, g=gsz)
                nc.vector.tensor_tensor(
                    ci_v[:, 0], cP[:, :, :, :, 0], cT[:, :, :, :, 1], mult)
                nc.vector.tensor_tensor(
                    ci_v[:, 1], cP[:, :, :, :, 1], cT[:, :, :, :, 0], mult)
                ci_r = cig[:, :2 * nlt * gsz * 16].rearrange(
                    "p (s lts g bc) -> p s lts bc g", s=2, lts=nlt, g=gsz)
                itmp = fin.tile([128, 96], fp32, tag="itmp", bufs=2)
                it = itmp[:, :2 * nlt * 16].rearrange(
                    "p (s lts bc) -> p s lts bc", s=2, lts=nlt)
                nc.vector.tensor_reduce(it, ci_r, axis=AX.X, op=add)
                if init:
                    nc.vector.tensor_tensor(tci[:, off:], it[:, 0], it[:, 1], sub)
                else:
                    nc.vector.tensor_tensor(
                        tci[:, off:], tci[:, off:], it[:, 0], add)
                    nc.vector.tensor_tensor(
                        tci[:, off:], tci[:, off:], it[:, 1], sub)
                if gi == 2:
                    # ---- launch AG-A while groups 3-6 still compute ----
                    nc.sync.dma_start(ar_in[:, :], ar_sb[:])
                    nc.gpsimd.collective_compute(
                        "AllGather", bypass,
                        replica_groups=[list(range(NCORES))],
                        ins=[ar_in[:, :]],
                        outs=[ar_out[:, :]],
                    )

            # ---- AG-B (lt1/lt2 remainder) + local tree-sums ----
            nc.sync.dma_start(ar_in2[:, :], ar2_sb[:])
            nc.gpsimd.collective_compute(
                "AllGather", bypass,
                replica_groups=[list(range(NCORES))],
                ins=[ar_in2[:, :]],
                outs=[ar_out2[:, :]],
            )
            gall = fin.tile([128, NCORES, 192], fp32)
            nc.sync.dma_start(
                gall[:], ar_out[:].rearrange("(r p) c -> p r c", r=NCORES))
            g4 = fin.tile([128, 4, 192], fp32)
            nc.vector.tensor_tensor(g4[:], gall[:, 0:4, :], gall[:, 4:8, :], add)
            g2 = fin.tile([128, 2, 192], fp32)
            nc.vector.tensor_tensor(g2[:], g4[:, 0:2, :], g4[:, 2:4, :], add)
            gA = fin.tile([128, 192], fp32)
            nc.vector.tensor_tensor(gA[:], g2[:, 0, :], g2[:, 1, :], add)

            # ---- final loss math, split by l-tile so the lt0 chain runs
            # while AG-B is still in flight.  legw carries sqrt(2) (the
            # reference's p = 2s scale) times LSC for fp8, so the gathered
            # sums are SC x reference; EPS consts scale to match and wvec
            # divides SC back out. ----
            EPS1 = EPS * SC
            EPS2 = EPS * SC * SC
            ps16 = fps.tile([16, 1], fp32, bufs=1)
            zb = fin.tile([128, 1], fp32)
            nc.vector.memset(zb[:], 0.0)
            e2b = fin.tile([128, 1], fp32)
            nc.vector.memset(e2b[:], EPS2)

            def emit_final(ppf, crf, cif, lt_lo, lt_hi, sfx):
                nl = lt_hi - lt_lo
                n16 = nl * 16
                ppb = fin.tile([128, nl * 32], fp32, name=f"ppb{sfx}")
                nc.vector.tensor_scalar(ppb[:], ppf, EPS1, None, add)
                ppt = ppb[:].rearrange("p (lt t bc) -> p lt t bc", lt=nl, t=T)
                p0 = ppt[:, :, 0, :]
                p1 = ppt[:, :, 1, :]
                sqp = fin.tile([128, nl * 32], fp32, name=f"sqp{sfx}")
                nc.scalar.activation(sqp[:], ppb[:], AF.Sqrt, bias=zb[:])
                sqv = sqp[:].rearrange("p (lt t bc) -> p lt t bc", lt=nl, t=T)
                d = fin.tile([128, n16], fp32, name=f"d{sfx}")
                nc.vector.tensor_tensor(
                    d[:].rearrange("p (lt bc) -> p lt bc", lt=nl),
                    sqv[:, :, 0, :], sqv[:, :, 1, :], sub)
                amp = fin.tile([128, n16], fp32, name=f"amp{sfx}")
                nc.vector.tensor_tensor(amp[:], d[:], d[:], mult)
                msr = fin.tile([128, n16], fp32, name=f"msr{sfx}")
                nc.vector.tensor_tensor(msr[:], crf, crf, mult)
                msi = fin.tile([128, n16], fp32, name=f"msi{sfx}")
                nc.vector.tensor_tensor(msi[:], cif, cif, mult)
                msum = fin.tile([128, n16], fp32, name=f"msum{sfx}")
                nc.vector.tensor_tensor(msum[:], msr[:], msi[:], add)
                mag = fin.tile([128, n16], fp32, name=f"mag{sfx}")
                nc.scalar.activation(mag[:], msum[:], AF.Sqrt, bias=zb[:])
                dprod = fin.tile([128, n16], fp32, name=f"dprod{sfx}")
                nc.vector.tensor_tensor(
                    dprod[:].rearrange("p (lt bc) -> p lt bc", lt=nl), p0, p1, mult)
                denom = fin.tile([128, n16], fp32, name=f"denom{sfx}")
                nc.scalar.activation(denom[:], dprod[:], AF.Sqrt, bias=e2b[:])
                dpe = fin.tile([128, n16], fp32, name=f"dpe{sfx}")
                nc.vector.tensor_scalar(dpe[:], denom[:], EPS1, None, add)
                rec = fin.tile([128, n16], fp32, name=f"rec{sfx}")
                nc.vector.reciprocal(rec[:], dpe[:])
                coh = fin.tile([128, n16], fp32, name=f"coh{sfx}")
                nc.vector.tensor_tensor(coh[:], mag[:], rec[:], mult)
                cohc = fin.tile([128, n16], fp32, name=f"cohc{sfx}")
                nc.vector.tensor_scalar(cohc[:], coh[:], 1.0, 0.0, amin, amax)
                mx = fin.tile([128, n16], fp32, name=f"mx{sfx}")
                nc.vector.tensor_tensor(
                    mx[:].rearrange("p (lt bc) -> p lt bc", lt=nl), p0, p1, amax)
                onemc = fin.tile([128, n16], fp32, name=f"onemc{sfx}")
                nc.vector.tensor_scalar(onemc[:], cohc[:], -1.0, 1.0, mult, add)
                dec = fin.tile([128, n16], fp32, name=f"dec{sfx}")
                nc.vector.scalar_tensor_tensor(dec[:], mx[:], 2.0, onemc[:], mult, mult)
                tot = fin.tile([128, n16], fp32, name=f"tot{sfx}")
                nc.vector.tensor_tensor(tot[:], dec[:], amp[:], add)
                totv = tot[:].rearrange("p (lt bc) -> p lt bc", lt=nl)
                for j, lt in enumerate(range(lt_lo, lt_hi)):
                    nc.tensor.matmul(ps16[:], totv[:, j, :], mask_sb[:, lt:lt + 1],
                                     start=(lt == 0), stop=(lt == LT - 1))

            # lt0 chain: complete after AG-A alone
            emit_final(gA[:, 0:32], gA[:, 96:112], gA[:, 144:160], 0, 1, "a")

            gallB = fin.tile([128, NCORES, 128], fp32)
            nc.sync.dma_start(
                gallB[:], ar_out2[:].rearrange("(r p) c -> p r c", r=NCORES))
            h4 = fin.tile([128, 4, 128], fp32)
            nc.vector.tensor_tensor(h4[:], gallB[:, 0:4, :], gallB[:, 4:8, :], add)
            h2 = fin.tile([128, 2, 128], fp32)
            nc.vector.tensor_tensor(h2[:], h4[:, 0:2, :], h4[:, 2:4, :], add)
            gB = fin.tile([128, 128], fp32)
            nc.vector.tensor_tensor(gB[:], h2[:, 0, :], h2[:, 1, :], add)
            g12 = fin.tile([128, 128], fp32)
            nc.vector.tensor_tensor(g12[:, 0:64], gA[:, 32:96], gB[:, 0:64], add)
            nc.vector.tensor_tensor(g12[:, 64:96], gA[:, 112:144], gB[:, 64:96], add)
            nc.vector.tensor_tensor(g12[:, 96:128], gA[:, 160:192], gB[:, 96:128], add)

            emit_final(g12[:, 0:64], g12[:, 64:96], g12[:, 96:128], 1, 3, "b")

            pc = fin.tile([16, 1], fp32)
            nc.vector.tensor_tensor(pc[:], ps16[:], wvec_sb[:], mult)
            ps1 = fps.tile([1, 1], fp32, bufs=1)
            nc.tensor.matmul(ps1[:], pc[:], ones_sb[:], start=True, stop=True)
            osb = fin.tile([1, 1], fp32)
            nc.any.tensor_copy(osb[:], ps1[:])
            nc.sync.dma_start(out_e[:, :], osb[:])

    nc.compile()
    return nc


def make_in_maps(prediction, target, weights, leg, w):
    if "tables" not in _CACHE:
        _CACHE["tables"] = _build_tables(leg, w, weights)
        _CACHE["w_id"] = np.asarray(weights, np.float32).copy()
    legw, dftc, wvec, ones16, lmask = _CACHE["tables"]
    if not np.array_equal(_CACHE["w_id"], np.asarray(weights, np.float32)):
        wvec = (np.tile(np.asarray(weights, np.float32), T) / (360.0 * 16.0 * SC)).reshape(16, 1)

    xTE, xTO = _pack_inputs(prediction, target)
    return [
        {
            "xT": xTE if cid % 2 == 0 else xTO,
            "legw": legw[cid],
            "dftT": dftc[cid],
            "wvec": wvec,
            "ones16": ones16,
            "lmask": lmask,
        }
        for cid in range(NCORES)
    ]


def kernel(prediction, target, weights, leg, w):
    from concourse.bass_utils import run_bass_kernel_spmd

    if "graph" not in _CACHE:
        _CACHE["graph"] = _build_graph()
    nc = _CACHE["graph"]

    in_maps = make_in_maps(prediction, target, weights, leg, w)
    res = run_bass_kernel_spmd(nc, in_maps, core_ids=list(range(NCORES)))
    out = np.asarray(res.results[0]["out"], np.float32).reshape(())
    return out



# revision 3
# speedup vs baseline: 1.2825x; 1.2825x over previous
"""Distributed Trainium2 Bass kernel for the spherical-harmonic AMSE loss.

Algorithm (8 NeuronCores, m-sharded; m = 8k + core_id interleave):
  host:    longitude fold — F_m = sum_{n<360} (x[n] + (-1)^m x[n+360]) w^{mn};
           every m on core cid has parity cid%2, so each core gets its own
           folded xT with per-parity sign.  xT is packed [120(p), 96(t,bc,jt),
           3(kt), 128(j)] fp8 so each DMA chunk is a linear per-partition run.
  stage 1: flipped DFT — xT tiles [120,128] are the PE *stationary*, the DFT
           twiddle block [120, kt, 92] bf16 is the moving operand.  Output
           F arrives already j-partitioned ([128 j', 92 m2] per (t,bc,jt)
           tile), so no DMA transpose is needed and the PE rolls straight
           into stage 2 (keeps the p-state ramp).
  stage 2: C[l', (m,t,bc,ri)] = legw.T @ FT   (PE, PSUM accum over j-tiles);
           legw streams fp8: k<16 on the gpsimd SW-DGE from t=0, the rest on
           the sync/scalar HW queues behind the xT chunks.
  stage 3: |C|^2 and conj(P)*T products + reductions over local m, chunked
           by m-group so it overlaps stage 2.  Products split across DVE and
           Pool; reductions on DVE; PSUM->SBUF copies on Scalar.
  Single AllGather of the merged bf16 partial payload [128, 192] (pp|cr|ci),
  local tree-sum, final loss math redundantly per core.
"""
import os
import numpy as np
import ml_dtypes

os.environ.setdefault("NEURON_RT_DBG_RDH_CC", "0")

NLON = 720
NLONF = 360          # folded longitude
L = 361
EPS = 1e-7
NCORES = 8
MSLOT = 46           # m slots per core (m = 8k + core_id; zero-padded if > 360)
M2 = 2 * MSLOT       # 92 live re/im columns
JP = 384             # padded latitude rows per (t, bc)  (3 * 128)
T = 2
BC = 16
KT = 3               # folded: 360 = 3 * 120
KTW = 120
LP = 384             # padded l (3 * 128)
LT = 3
TILES = T * BC * 3   # 96 (t, bc, jt) stationary tiles
TPC = 8              # tiles per DMA chunk
NCH = TILES // TPC   # 12 chunks
TPB = 4              # tiles per PSUM bank in stage 1
NB = TILES // TPB    # 24 banks

bf16 = ml_dtypes.bfloat16
f8 = ml_dtypes.float8_e4m3
LSC = 256.0              # fp8 legw pre-scale (keeps values in normal range)
SC = LSC * LSC           # pp/cr/ci come out scaled by LSC^2; wvec unwinds it

_CACHE = {}


def _build_tables(leg, w, weights):
    legf = np.asarray(leg, np.float32)          # [L, M, J]
    wf = np.asarray(w, np.float32)              # [J]
    legT = legf.transpose(1, 2, 0) * wf[None, :, None]   # [M, J, L]
    legT[0] *= np.float32(2.0 ** -0.5)          # uniform p = 2*sum|C|^2
    legT *= np.float32(2.0 ** 0.5)              # bake the psd 2x
    legT *= np.float32(LSC)                     # fp8 normal-range pre-scale
    legp = np.zeros((MSLOT * NCORES, JP, LP), np.float32)
    legp[:L, :L, :L] = legT
    legp = legp.reshape(MSLOT, NCORES, JP, LP).transpose(1, 0, 2, 3)
    legw = np.ascontiguousarray(
        legp.reshape(NCORES, MSLOT, 3, 128, LP).transpose(0, 1, 3, 2, 4)
    ).astype(f8)                                # [8][46, 128(j'), 3(jt), 384(l)]

    n = np.arange(NLONF, dtype=np.float64)
    m_all = np.arange(MSLOT * NCORES, dtype=np.float64)
    ang = 2.0 * np.pi * np.outer(n, m_all) / NLON
    scale = 2.0 * np.pi / NLON
    dft = np.zeros((NLONF, MSLOT * NCORES, 2), np.float64)
    dft[:, :, 0] = np.cos(ang) * scale
    dft[:, :, 1] = -np.sin(ang) * scale
    dft[:, L:, :] = 0.0
    dft = dft.reshape(NLONF, MSLOT, NCORES, 2).transpose(2, 0, 1, 3)  # [8,360,46,2]
    dft = dft.reshape(NCORES, KT, KTW, M2).transpose(0, 2, 1, 3)      # [8,120,3,92]
    dftc = np.ascontiguousarray(dft).astype(bf16)

    wvec = (np.tile(np.asarray(weights, np.float32), T) / (360.0 * 16.0 * SC)).reshape(16, 1)
    ones16 = np.ones((16, 1), np.float32)
    lmask = np.zeros((128, LT), np.float32)
    for lt in range(LT):
        for p in range(128):
            if lt * 128 + p < L - 1:
                lmask[p, lt] = 1.0
    return legw, dftc, wvec, ones16, lmask


def _pack_inputs(prediction, target):
    x = np.zeros((T, BC, JP, NLON), np.float32)
    x[0, :, :L] = np.asarray(prediction, np.float32).reshape(BC, L, NLON)
    x[1, :, :L] = np.asarray(target, np.float32).reshape(BC, L, NLON)
    lo, hi = x[..., :NLONF], x[..., NLONF:]

    def pack(xf):
        # xf [T, BC, JP, 360] -> [120(p), (t bc jt), kt, 128(j)]
        a = xf.transpose(3, 0, 1, 2)                 # [360, T, BC, JP]
        a = a.reshape(KT, KTW, T, BC, 3, 128)        # [kt, p, t, bc, jt, jc]
        a = a.transpose(1, 2, 3, 4, 0, 5)            # [p, t, bc, jt, kt, jc]
        return np.ascontiguousarray(a.reshape(KTW, TILES, KT, 128)).astype(f8)

    return pack(lo + hi), pack(lo - hi)


def _build_graph():
    import concourse.bacc as bacc
    import concourse.mybir as mybir
    from concourse.tile import TileContext

    fp32 = mybir.dt.float32
    bft = mybir.dt.bfloat16
    f8t = mybir.dt.float8e4

    nc = bacc.Bacc(None, target_bir_lowering=False)

    xT_e = nc.declare_dram_parameter("xT", [KTW, TILES, KT, 128], f8t, isOutput=False)
    legw_e = nc.declare_dram_parameter("legw", [MSLOT, 128, 3, LP], f8t, isOutput=False)
    dft_e = nc.declare_dram_parameter("dftT", [KTW, KT, M2], bft, isOutput=False)
    wvec_e = nc.declare_dram_parameter("wvec", [16, 1], fp32, isOutput=False)
    ones_e = nc.declare_dram_parameter("ones16", [16, 1], fp32, isOutput=False)
    mask_e = nc.declare_dram_parameter("lmask", [128, LT], fp32, isOutput=False)
    out_e = nc.declare_dram_parameter("out", [1, 1], fp32, isOutput=True)

    ar_in = nc.dram_tensor("ar_in", [128, 192], bft)
    ar_out = nc.dram_tensor("ar_out", [NCORES * 128, 192], bft, addr_space="Shared")

    add = mybir.AluOpType.add
    sub = mybir.AluOpType.subtract
    mult = mybir.AluOpType.mult
    amax = mybir.AluOpType.max
    amin = mybir.AluOpType.min
    bypass = mybir.AluOpType.bypass
    AF = mybir.ActivationFunctionType
    AX = mybir.AxisListType

    with TileContext(nc) as tc:
        with (
            tc.tile_pool(name="consts", bufs=1) as consts,
            tc.tile_pool(name="xp", bufs=NCH) as xp,
            tc.tile_pool(name="fps", bufs=4, space="PSUM") as fps,
            tc.tile_pool(name="big", bufs=1) as big,
            tc.tile_pool(name="lw", bufs=32) as lwp,
            tc.tile_pool(name="cps", bufs=3, space="PSUM") as cps,
            tc.tile_pool(name="fin", bufs=1) as fin,
        ):
            dft_sb = consts.tile([KTW, KT, M2], bft)
            nc.sync.dma_start(dft_sb[:], dft_e[:])
            wvec_sb = consts.tile([16, 1], fp32)
            nc.sync.dma_start(wvec_sb[:], wvec_e[:])
            ones_sb = consts.tile([16, 1], fp32)
            nc.sync.dma_start(ones_sb[:], ones_e[:])
            mask_sb = consts.tile([128, LT], fp32)
            nc.sync.dma_start(mask_sb[:], mask_e[:])

            # ---- legw streaming: k<16 rides the gpsimd SW-DGE from t=0 so
            # that queue is drained well before the collective triggers; the
            # rest ride the sync/scalar HW queues behind the xT chunks ----
            MGROUPS = [(0, 8), (8, 16), (16, 24), (24, 32), (32, 38), (38, 43), (43, 46)]
            LTMIN = [0, 0, 1, 1, 2, 2, 2]
            lw_tiles = {}

            def load_lw(k, eng):
                lt0 = min(2, k // 16)   # l-tiles below the diagonal are zero
                lw = lwp.tile([128, 3, LP], f8t, name="lw")
                eng.dma_start(lw[:, :, lt0 * 128:LP],
                              legw_e[k][:, :, lt0 * 128:LP])
                lw_tiles[k] = lw

            for k in range(16):
                load_lw(k, nc.gpsimd)

            # ---- stage 1: flipped DFT ----
            FT_sb = big.tile([128, TILES * M2], bft)     # [j', (t bc jt m2)]
            xch = []
            for g in range(NCH):
                xt = xp.tile([KTW, TPC, KT, 128], f8t, name="xch")
                eng = nc.sync if g % 2 == 0 else nc.scalar
                eng.dma_start(xt[:], xT_e[:, g * TPC:(g + 1) * TPC, :, :])
                xch.append(xt)
            for b in range(NB):
                ps = fps.tile([128, 512], fp32, tag="s1", bufs=3)
                for i in range(TPB):
                    tl = b * TPB + i
                    xt = xch[tl // TPC]
                    for kt in range(KT):
                        nc.tensor.matmul(
                            ps[:, i * M2:(i + 1) * M2],
                            xt[:, tl % TPC, kt, :],
                            dft_sb[:, kt, :],
                            start=(kt == 0), stop=(kt == KT - 1),
                        )
                dst = FT_sb[:, b * TPB * M2:(b + 1) * TPB * M2]
                if b % 2 == 0:
                    nc.scalar.activation(dst, ps[:, :TPB * M2], AF.Copy)
                else:
                    nc.vector.tensor_copy(dst, ps[:, :TPB * M2])
                # interleave the remaining legw loads behind the xT chunks
                if b == 0:
                    for k in range(16, 24):
                        load_lw(k, nc.sync if k % 2 == 0 else nc.scalar)
                if b == 2:
                    for k in range(24, 34):
                        load_lw(k, nc.sync if k % 2 == 0 else nc.scalar)
                if b == 4:
                    for k in range(34, 46):
                        load_lw(k, nc.sync if k % 2 == 0 else nc.scalar)
            FT_v = FT_sb[:].rearrange(
                "p (t bc jt m) -> p t bc jt m", t=T, bc=BC, jt=3, m=M2
            )

            # ---- stage 2 + stage 3 per m-group ----
            # Call holds all C tiles [p, (lt, k, t, bc, ri)] at absolute m
            # slots so stage-3 ops can span a group's full lt range.
            Call = big.tile([128, LT * MSLOT * 64], bft)
            Cv_all = Call[:].rearrange(
                "p (lt k t bc ri) -> p lt k t bc ri", lt=LT, k=MSLOT, t=T, bc=BC)
            # merged all-reduce payload: [0:96] pp (lt,t,bc) | [96:144] cr
            # (lt,bc) | [144:192] ci (lt,bc)
            ar_sb = fin.tile([128, 192], fp32)
            ar_pp = ar_sb[:, 0:96].rearrange("p (lt t bc) -> p lt t bc", lt=LT, t=T)
            ar_cr = ar_sb[:, 96:144].rearrange("p (lt bc) -> p lt bc", lt=LT)
            ar_ci = ar_sb[:, 144:192].rearrange("p (lt bc) -> p lt bc", lt=LT)
            for gi, (m0, m1) in enumerate(MGROUPS):
                gsz = m1 - m0
                ltm = LTMIN[gi]
                nlt = LT - ltm
                init = gi == 0
                for lt in range(ltm, LT):
                    ps = cps.tile([128, 512], fp32)
                    for mi in range(gsz):
                        k = m0 + mi
                        rhs = FT_v[:, :, :, :, 2 * k:2 * k + 2]
                        for jt in range(3):
                            nc.tensor.matmul(
                                ps[:, mi * 64:(mi + 1) * 64],
                                lw_tiles[k][:, jt, lt * 128:(lt + 1) * 128],
                                rhs[:, :, :, jt, :],
                                start=(jt == 0), stop=(jt == 2),
                            )
                    nc.scalar.activation(
                        Call[:, (lt * MSLOT + m0) * 64:(lt * MSLOT + m1) * 64],
                        ps[:, 0:gsz * 64], AF.Copy)
                # ---- stage 3 for this m-group (overlaps next group's matmuls).
                # products: sq on DVE, cr/ci on Pool; reductions on DVE ----
                Cv = Cv_all[:, ltm:, m0:m1]   # [p, lts, g, t, bc, ri]
                cP = Cv[:, :, :, 0, :, :]     # [p, lts, g, bc, ri]
                cT = Cv[:, :, :, 1, :, :]
                sqg = fin.tile([128, LT * 8 * 64], bft, tag="sqg", bufs=2)
                sq_v = sqg[:, :nlt * gsz * 64].rearrange(
                    "p (lts g t bc ri) -> p lts g t bc ri", lts=nlt, g=gsz, t=T, bc=BC)
                nc.vector.tensor_tensor(sq_v, Cv, Cv, mult)
                sq_r = sqg[:, :nlt * gsz * 64].rearrange(
                    "p (lts g t bc ri) -> p lts t bc g ri", lts=nlt, g=gsz, t=T, bc=BC)
                if init:
                    nc.vector.tensor_reduce(ar_pp[:, ltm:], sq_r, axis=AX.XY, op=add)
                else:
                    rtmp = fin.tile([128, 96], fp32, tag="rtmp", bufs=2)
                    rt = rtmp[:, :nlt * 32].rearrange(
                        "p (lts t bc) -> p lts t bc", lts=nlt, t=T)
                    nc.vector.tensor_reduce(rt, sq_r, axis=AX.XY, op=add)
                    nc.vector.tensor_tensor(
                        ar_pp[:, ltm:], ar_pp[:, ltm:], rt, add)
                crg = fin.tile([128, LT * 8 * 32], bft, tag="crg", bufs=2)
                cr_v = crg[:, :nlt * gsz * 32].rearrange(
                    "p (lts g bc ri) -> p lts g bc ri", lts=nlt, g=gsz, bc=BC)
                nc.gpsimd.tensor_tensor(cr_v, cP, cT, mult)
                cr_r = crg[:, :nlt * gsz * 32].rearrange(
                    "p (lts g bc ri) -> p lts bc g ri", lts=nlt, g=gsz, bc=BC)
                if init:
                    nc.vector.tensor_reduce(ar_cr[:, ltm:], cr_r, axis=AX.XY, op=add)
                else:
                    ctmp = fin.tile([128, 48], fp32, tag="ctmp", bufs=2)
                    ct = ctmp[:, :nlt * 16].rearrange("p (lts bc) -> p lts bc", lts=nlt)
                    nc.vector.tensor_reduce(ct, cr_r, axis=AX.XY, op=add)
                    nc.vector.tensor_tensor(
                        ar_cr[:, ltm:], ar_cr[:, ltm:], ct, add)
                cig = fin.tile([128, 2 * LT * 8 * 16], bft, tag="cig", bufs=2)
                ci_v = cig[:, :2 * nlt * gsz * 16].rearrange(
                    "p (s lts g bc) -> p s lts g bc", s=2, lts=nlt, g=gsz)
                nc.gpsimd.tensor_tensor(
                    ci_v[:, 0], cP[:, :, :, :, 0], cT[:, :, :, :, 1], mult)
                nc.gpsimd.tensor_tensor(
                    ci_v[:, 1], cP[:, :, :, :, 1], cT[:, :, :, :, 0], mult)
                ci_r = cig[:, :2 * nlt * gsz * 16].rearrange(
                    "p (s lts g bc) -> p s lts bc g", s=2, lts=nlt, g=gsz)
                itmp = fin.tile([128, 96], fp32, tag="itmp", bufs=2)
                it = itmp[:, :2 * nlt * 16].rearrange(
                    "p (s lts bc) -> p s lts bc", s=2, lts=nlt)
                nc.vector.tensor_reduce(it, ci_r, axis=AX.X, op=add)
                if init:
                    nc.vector.tensor_tensor(ar_ci[:, ltm:], it[:, 0], it[:, 1], sub)
                else:
                    nc.vector.tensor_tensor(
                        ar_ci[:, ltm:], ar_ci[:, ltm:], it[:, 0], add)
                    nc.vector.tensor_tensor(
                        ar_ci[:, ltm:], ar_ci[:, ltm:], it[:, 1], sub)

            # ---- single AllGather of the bf16 payload + local tree-sum ----
            arh = fin.tile([128, 192], bft)
            nc.vector.tensor_copy(arh[:], ar_sb[:])
            nc.sync.dma_start(ar_in[:, :], arh[:])
            nc.gpsimd.collective_compute(
                "AllGather", bypass,
                replica_groups=[list(range(NCORES))],
                ins=[ar_in[:, :]],
                outs=[ar_out[:, :]],
            )
            gall = fin.tile([128, NCORES, 192], bft)
            nc.sync.dma_start(
                gall[:], ar_out[:].rearrange("(r p) c -> p r c", r=NCORES))
            g4 = fin.tile([128, 4, 192], fp32)
            nc.vector.tensor_tensor(g4[:], gall[:, 0:4, :], gall[:, 4:8, :], add)
            g2 = fin.tile([128, 2, 192], fp32)
            nc.vector.tensor_tensor(g2[:], g4[:, 0:2, :], g4[:, 2:4, :], add)
            gA = fin.tile([128, 192], fp32)
            nc.vector.tensor_tensor(gA[:], g2[:, 0, :], g2[:, 1, :], add)

            # ---- final loss math.  legw carries sqrt(2) (the reference's
            # p = 2s scale) times LSC for fp8, so the gathered sums are
            # SC x reference; EPS consts scale to match and wvec divides SC
            # back out. ----
            EPS1 = EPS * SC
            EPS2 = EPS * SC * SC
            ps16 = fps.tile([16, 1], fp32, tag="ps16", bufs=1)
            zb = fin.tile([128, 1], fp32)
            nc.vector.memset(zb[:], 0.0)
            e2b = fin.tile([128, 1], fp32)
            nc.vector.memset(e2b[:], EPS2)

            def emit_final(ppf, crf, cif, lt_lo, lt_hi, sfx):
                nl = lt_hi - lt_lo
                n16 = nl * 16
                ppb = fin.tile([128, nl * 32], fp32, name=f"ppb{sfx}")
                nc.vector.tensor_scalar(ppb[:], ppf, EPS1, None, add)
                ppt = ppb[:].rearrange("p (lt t bc) -> p lt t bc", lt=nl, t=T)
                p0 = ppt[:, :, 0, :]
                p1 = ppt[:, :, 1, :]
                sqp = fin.tile([128, nl * 32], fp32, name=f"sqp{sfx}")
                nc.scalar.activation(sqp[:], ppb[:], AF.Sqrt, bias=zb[:])
                sqv = sqp[:].rearrange("p (lt t bc) -> p lt t bc", lt=nl, t=T)
                d = fin.tile([128, n16], fp32, name=f"d{sfx}")
                nc.vector.tensor_tensor(
                    d[:].rearrange("p (lt bc) -> p lt bc", lt=nl),
                    sqv[:, :, 0, :], sqv[:, :, 1, :], sub)
                amp = fin.tile([128, n16], fp32, name=f"amp{sfx}")
                nc.vector.tensor_tensor(amp[:], d[:], d[:], mult)
                msr = fin.tile([128, n16], fp32, name=f"msr{sfx}")
                nc.vector.tensor_tensor(msr[:], crf, crf, mult)
                msi = fin.tile([128, n16], fp32, name=f"msi{sfx}")
                nc.vector.tensor_tensor(msi[:], cif, cif, mult)
                msum = fin.tile([128, n16], fp32, name=f"msum{sfx}")
                nc.vector.tensor_tensor(msum[:], msr[:], msi[:], add)
                mag = fin.tile([128, n16], fp32, name=f"mag{sfx}")
                nc.scalar.activation(mag[:], msum[:], AF.Sqrt, bias=zb[:])
                dprod = fin.tile([128, n16], fp32, name=f"dprod{sfx}")
                nc.vector.tensor_tensor(
                    dprod[:].rearrange("p (lt bc) -> p lt bc", lt=nl), p0, p1, mult)
                denom = fin.tile([128, n16], fp32, name=f"denom{sfx}")
                nc.scalar.activation(denom[:], dprod[:], AF.Sqrt, bias=e2b[:])
                dpe = fin.tile([128, n16], fp32, name=f"dpe{sfx}")
                nc.vector.tensor_scalar(dpe[:], denom[:], EPS1, None, add)
                rec = fin.tile([128, n16], fp32, name=f"rec{sfx}")
                nc.vector.reciprocal(rec[:], dpe[:])
                coh = fin.tile([128, n16], fp32, name=f"coh{sfx}")
                nc.vector.tensor_tensor(coh[:], mag[:], rec[:], mult)
                cohc = fin.tile([128, n16], fp32, name=f"cohc{sfx}")
                nc.vector.tensor_scalar(cohc[:], coh[:], 1.0, 0.0, amin, amax)
                mx = fin.tile([128, n16], fp32, name=f"mx{sfx}")
                nc.vector.tensor_tensor(
                    mx[:].rearrange("p (lt bc) -> p lt bc", lt=nl), p0, p1, amax)
                onemc = fin.tile([128, n16], fp32, name=f"onemc{sfx}")
                nc.vector.tensor_scalar(onemc[:], cohc[:], -1.0, 1.0, mult, add)
                dec = fin.tile([128, n16], fp32, name=f"dec{sfx}")
                nc.vector.scalar_tensor_tensor(dec[:], mx[:], 2.0, onemc[:], mult, mult)
                tot = fin.tile([128, n16], fp32, name=f"tot{sfx}")
                nc.vector.tensor_tensor(tot[:], dec[:], amp[:], add)
                totv = tot[:].rearrange("p (lt bc) -> p lt bc", lt=nl)
                for j, lt in enumerate(range(lt_lo, lt_hi)):
                    nc.tensor.matmul(ps16[:], totv[:, j, :], mask_sb[:, lt:lt + 1],
                                     start=(lt == 0), stop=(lt == LT - 1))

            emit_final(gA[:, 0:96].rearrange("p (lt t bc) -> p lt t bc", lt=LT, t=T),
                       gA[:, 96:144].rearrange("p (lt bc) -> p lt bc", lt=LT),
                       gA[:, 144:192].rearrange("p (lt bc) -> p lt bc", lt=LT),
                       0, LT, "a")

            pc = fin.tile([16, 1], fp32)
            nc.vector.tensor_tensor(pc[:], ps16[:], wvec_sb[:], mult)
            ps1 = fps.tile([1, 1], fp32, tag="ps1", bufs=1)
            nc.tensor.matmul(ps1[:], pc[:], ones_sb[:], start=True, stop=True)
            osb = fin.tile([1, 1], fp32)
            nc.any.tensor_copy(osb[:], ps1[:])
            nc.sync.dma_start(out_e[:, :], osb[:])

    nc.compile()
    return nc


def make_in_maps(prediction, target, weights, leg, w):
    if "tables" not in _CACHE:
        _CACHE["tables"] = _build_tables(leg, w, weights)
        _CACHE["w_id"] = np.asarray(weights, np.float32).copy()
    legw, dftc, wvec, ones16, lmask = _CACHE["tables"]
    if not np.array_equal(_CACHE["w_id"], np.asarray(weights, np.float32)):
        wvec = (np.tile(np.asarray(weights, np.float32), T) / (360.0 * 16.0 * SC)).reshape(16, 1)

    xTE, xTO = _pack_inputs(prediction, target)
    return [
        {
            "xT": xTE if cid % 2 == 0 else xTO,
            "legw": legw[cid],
            "dftT": dftc[cid],
            "wvec": wvec,
            "ones16": ones16,
            "lmask": lmask,
        }
        for cid in range(NCORES)
    ]


def kernel(prediction, target, weights, leg, w):
    from concourse.bass_utils import run_bass_kernel_spmd

    if "graph" not in _CACHE:
        _CACHE["graph"] = _build_graph()
    nc = _CACHE["graph"]

    in_maps = make_in_maps(prediction, target, weights, leg, w)
    res = run_bass_kernel_spmd(nc, in_maps, core_ids=list(range(NCORES)))
    out = np.asarray(res.results[0]["out"], np.float32).reshape(())
    return out


# revision 9
# speedup vs baseline: 1.2978x; 1.0119x over previous
"""Distributed Trainium2 Bass kernel for the spherical-harmonic AMSE loss.

Algorithm (8 NeuronCores, m-sharded; m = 8k + core_id interleave):
  host:    longitude fold — F_m = sum_{n<360} (x[n] + (-1)^m x[n+360]) w^{mn};
           every m on core cid has parity cid%2, so each core gets its own
           folded xT with per-parity sign.  xT is packed [120(p), 96(t,bc,jt),
           3(kt), 128(j)] fp8 so each DMA chunk is a linear per-partition run.
  stage 1: flipped DFT — xT tiles [120,128] are the PE *stationary*, the DFT
           twiddle block [120, kt, 92] bf16 is the moving operand.  Output
           F arrives already j-partitioned ([128 j', 92 m2] per (t,bc,jt)
           tile), so no DMA transpose is needed and the PE rolls straight
           into stage 2 (keeps the p-state ramp).
  stage 2: C[l', (m,t,bc,ri)] = legw.T @ FT   (PE, PSUM accum over j-tiles);
           legw streams fp8: k<16 on the gpsimd SW-DGE from t=0, the rest on
           the sync/scalar HW queues behind the xT chunks.
  stage 3: |C|^2 and conj(P)*T products + reductions over local m, chunked
           by m-group so it overlaps stage 2.  Products split across DVE and
           Pool; reductions on DVE; PSUM->SBUF copies on Scalar.
  Single AllGather of the merged bf16 partial payload [128, 192] (pp|cr|ci),
  local tree-sum, final loss math redundantly per core.
"""
import os
import numpy as np
import ml_dtypes

os.environ.setdefault("NEURON_RT_DBG_RDH_CC", "0")

NLON = 720
NLONF = 360          # folded longitude
L = 361
EPS = 1e-7
NCORES = 8
MSLOT = 46           # m slots per core (m = 8k + core_id; zero-padded if > 360)
M2 = 2 * MSLOT       # 92 live re/im columns
JP = 384             # padded latitude rows per (t, bc)  (3 * 128)
T = 2
BC = 16
KT = 3               # folded: 360 = 3 * 120
KTW = 120
LP = 384             # padded l (3 * 128)
LT = 3
TILES = T * BC * 3   # 96 (t, bc, jt) stationary tiles
TPC = 8              # tiles per DMA chunk
NCH = TILES // TPC   # 12 chunks
TPB = 4              # tiles per PSUM bank in stage 1
NB = TILES // TPB    # 24 banks

bf16 = ml_dtypes.bfloat16
f8 = ml_dtypes.float8_e4m3
LSC = 256.0              # fp8 legw pre-scale (keeps values in normal range)
SC = LSC * LSC           # pp/cr/ci come out scaled by LSC^2; wvec unwinds it

_CACHE = {}


def _build_tables(leg, w, weights):
    legf = np.asarray(leg, np.float32)          # [L, M, J]
    wf = np.asarray(w, np.float32)              # [J]
    legT = legf.transpose(1, 2, 0) * wf[None, :, None]   # [M, J, L]
    legT[0] *= np.float32(2.0 ** -0.5)          # uniform p = 2*sum|C|^2
    legT *= np.float32(2.0 ** 0.5)              # bake the psd 2x
    legT *= np.float32(LSC)                     # fp8 normal-range pre-scale
    legp = np.zeros((MSLOT * NCORES, JP, LP), np.float32)
    legp[:L, :L, :L] = legT
    legp = legp.reshape(MSLOT, NCORES, JP, LP).transpose(1, 0, 2, 3)
    legw = np.ascontiguousarray(
        legp.reshape(NCORES, MSLOT, 3, 128, LP).transpose(0, 1, 3, 2, 4)
    ).astype(f8)                                # [8][46, 128(j'), 3(jt), 384(l)]

    n = np.arange(NLONF, dtype=np.float64)
    m_all = np.arange(MSLOT * NCORES, dtype=np.float64)
    ang = 2.0 * np.pi * np.outer(n, m_all) / NLON
    scale = 2.0 * np.pi / NLON
    dft = np.zeros((NLONF, MSLOT * NCORES, 2), np.float64)
    dft[:, :, 0] = np.cos(ang) * scale
    dft[:, :, 1] = -np.sin(ang) * scale
    dft[:, L:, :] = 0.0
    dft = dft.reshape(NLONF, MSLOT, NCORES, 2).transpose(2, 0, 1, 3)  # [8,360,46,2]
    dft = dft.reshape(NCORES, KT, KTW, M2).transpose(0, 2, 1, 3)      # [8,120,3,92]
    dftc = np.ascontiguousarray(dft).astype(bf16)

    wvec = (np.tile(np.asarray(weights, np.float32), T) / (360.0 * 16.0 * SC)).reshape(16, 1)
    ones16 = np.ones((16, 1), np.float32)
    lmask = np.zeros((128, LT), np.float32)
    for lt in range(LT):
        for p in range(128):
            if lt * 128 + p < L - 1:
                lmask[p, lt] = 1.0
    return legw, dftc, wvec, ones16, lmask


def _pack_inputs(prediction, target):
    x = np.zeros((T, BC, JP, NLON), np.float32)
    x[0, :, :L] = np.asarray(prediction, np.float32).reshape(BC, L, NLON)
    x[1, :, :L] = np.asarray(target, np.float32).reshape(BC, L, NLON)
    lo, hi = x[..., :NLONF], x[..., NLONF:]

    def pack(xf):
        # xf [T, BC, JP, 360] -> [120(p), (t bc jt), kt, 128(j)]
        a = xf.transpose(3, 0, 1, 2)                 # [360, T, BC, JP]
        a = a.reshape(KT, KTW, T, BC, 3, 128)        # [kt, p, t, bc, jt, jc]
        a = a.transpose(1, 2, 3, 4, 0, 5)            # [p, t, bc, jt, kt, jc]
        return np.ascontiguousarray(a.reshape(KTW, TILES, KT, 128)).astype(f8)

    return pack(lo + hi), pack(lo - hi)


def _build_graph():
    import concourse.bacc as bacc
    import concourse.mybir as mybir
    from concourse.tile import TileContext

    fp32 = mybir.dt.float32
    bft = mybir.dt.bfloat16
    f8t = mybir.dt.float8e4

    nc = bacc.Bacc(None, target_bir_lowering=False)

    xT_e = nc.declare_dram_parameter("xT", [KTW, TILES, KT, 128], f8t, isOutput=False)
    legw_e = nc.declare_dram_parameter("legw", [MSLOT, 128, 3, LP], f8t, isOutput=False)
    dft_e = nc.declare_dram_parameter("dftT", [KTW, KT, M2], bft, isOutput=False)
    wvec_e = nc.declare_dram_parameter("wvec", [16, 1], fp32, isOutput=False)
    ones_e = nc.declare_dram_parameter("ones16", [16, 1], fp32, isOutput=False)
    mask_e = nc.declare_dram_parameter("lmask", [128, LT], fp32, isOutput=False)
    out_e = nc.declare_dram_parameter("out", [1, 1], fp32, isOutput=True)

    ar_in = nc.dram_tensor("ar_in", [128, 192], bft)
    ar_out = nc.dram_tensor("ar_out", [NCORES * 128, 192], bft, addr_space="Shared")
    wu_in = nc.dram_tensor("wu_in", [128, 2], bft)
    wu_out = nc.dram_tensor("wu_out", [NCORES * 128, 2], bft, addr_space="Shared")

    add = mybir.AluOpType.add
    sub = mybir.AluOpType.subtract
    mult = mybir.AluOpType.mult
    amax = mybir.AluOpType.max
    amin = mybir.AluOpType.min
    bypass = mybir.AluOpType.bypass
    AF = mybir.ActivationFunctionType
    AX = mybir.AxisListType

    with TileContext(nc) as tc:
        with (
            tc.tile_pool(name="consts", bufs=1) as consts,
            tc.tile_pool(name="xp", bufs=NCH) as xp,
            tc.tile_pool(name="fps", bufs=4, space="PSUM") as fps,
            tc.tile_pool(name="big", bufs=1) as big,
            tc.tile_pool(name="lw", bufs=32) as lwp,
            tc.tile_pool(name="cps", bufs=3, space="PSUM") as cps,
            tc.tile_pool(name="fin", bufs=1) as fin,
        ):
            # ---- warm-up collective: absorbs the RDH first-collective
            # setup + barrier so the real AllGather runs warm ----
            wu_sb = consts.tile([128, 2], bft)
            nc.vector.memset(wu_sb[:], 0.0)
            nc.gpsimd.dma_start(wu_in[:, :], wu_sb[:])
            nc.gpsimd.collective_compute(
                "AllGather", mybir.AluOpType.bypass,
                replica_groups=[list(range(NCORES))],
                ins=[wu_in[:, :]],
                outs=[wu_out[:, :]],
            )

            dft_sb = consts.tile([KTW, KT, M2], bft)
            nc.sync.dma_start(dft_sb[:], dft_e[:])
            wvec_sb = consts.tile([16, 1], fp32)
            nc.sync.dma_start(wvec_sb[:], wvec_e[:])
            ones_sb = consts.tile([16, 1], fp32)
            nc.sync.dma_start(ones_sb[:], ones_e[:])
            mask_sb = consts.tile([128, LT], fp32)
            nc.sync.dma_start(mask_sb[:], mask_e[:])

            # ---- legw streaming: k<16 rides the gpsimd SW-DGE from t=0 so
            # that queue is drained well before the collective triggers; the
            # rest ride the sync/scalar HW queues behind the xT chunks ----
            MGROUPS = [(0, 8), (8, 16), (16, 24), (24, 32), (32, 38), (38, 43), (43, 46)]
            LTMIN = [0, 0, 1, 1, 2, 2, 2]
            lw_tiles = {}

            def load_lw(k, eng):
                lt0 = min(2, k // 16)   # l-tiles below the diagonal are zero
                lw = lwp.tile([128, 3, LP], f8t, name="lw")
                eng.dma_start(lw[:, :, lt0 * 128:LP],
                              legw_e[k][:, :, lt0 * 128:LP])
                lw_tiles[k] = lw

            for k in range(28):
                load_lw(k, nc.gpsimd)

            # ---- stage 1: flipped DFT ----
            FT_sb = big.tile([128, TILES * M2], bft)     # [j', (t bc jt m2)]
            xch = []
            for g in range(NCH):
                xt = xp.tile([KTW, TPC, KT, 128], f8t, name="xch")
                eng = nc.sync if g % 2 == 0 else nc.scalar
                eng.dma_start(xt[:], xT_e[:, g * TPC:(g + 1) * TPC, :, :])
                xch.append(xt)
            for k in range(28, MSLOT):
                load_lw(k, nc.sync if k % 2 == 0 else nc.scalar)
            for b in range(NB):
                ps = fps.tile([128, 512], fp32, tag="s1", bufs=3)
                for i in range(TPB):
                    tl = b * TPB + i
                    xt = xch[tl // TPC]
                    for kt in range(KT):
                        nc.tensor.matmul(
                            ps[:, i * M2:(i + 1) * M2],
                            xt[:, tl % TPC, kt, :],
                            dft_sb[:, kt, :],
                            start=(kt == 0), stop=(kt == KT - 1),
                        )
                dst = FT_sb[:, b * TPB * M2:(b + 1) * TPB * M2]
                if b % 2 == 0:
                    nc.scalar.activation(dst, ps[:, :TPB * M2], AF.Copy)
                else:
                    nc.vector.tensor_copy(dst, ps[:, :TPB * M2])
            FT_v = FT_sb[:].rearrange(
                "p (t bc jt m) -> p t bc jt m", t=T, bc=BC, jt=3, m=M2
            )

            # ---- stage 2 + stage 3 per m-group ----
            # Call holds all C tiles [p, (lt, k, t, bc, ri)] at absolute m
            # slots so stage-3 ops can span a group's full lt range.
            Call = big.tile([128, LT * MSLOT * 64], bft)
            Cv_all = Call[:].rearrange(
                "p (lt k t bc ri) -> p lt k t bc ri", lt=LT, k=MSLOT, t=T, bc=BC)
            # merged all-reduce payload: [0:96] pp (lt,t,bc) | [96:144] cr
            # (lt,bc) | [144:192] ci (lt,bc)
            ar_sb = fin.tile([128, 192], fp32)
            ar_pp = ar_sb[:, 0:96].rearrange("p (lt t bc) -> p lt t bc", lt=LT, t=T)
            ar_cr = ar_sb[:, 96:144].rearrange("p (lt bc) -> p lt bc", lt=LT)
            ar_ci = ar_sb[:, 144:192].rearrange("p (lt bc) -> p lt bc", lt=LT)
            for gi, (m0, m1) in enumerate(MGROUPS):
                gsz = m1 - m0
                ltm = LTMIN[gi]
                nlt = LT - ltm
                init = gi == 0
                for lt in range(ltm, LT):
                    ps = cps.tile([128, 512], fp32)
                    for mi in range(gsz):
                        k = m0 + mi
                        rhs = FT_v[:, :, :, :, 2 * k:2 * k + 2]
                        for jt in range(3):
                            nc.tensor.matmul(
                                ps[:, mi * 64:(mi + 1) * 64],
                                lw_tiles[k][:, jt, lt * 128:(lt + 1) * 128],
                                rhs[:, :, :, jt, :],
                                start=(jt == 0), stop=(jt == 2),
                            )
                    nc.scalar.activation(
                        Call[:, (lt * MSLOT + m0) * 64:(lt * MSLOT + m1) * 64],
                        ps[:, 0:gsz * 64], AF.Copy)
                # ---- stage 3 for this m-group (overlaps next group's matmuls).
                # products: sq on DVE, cr/ci on Pool; reductions on DVE ----
                Cv = Cv_all[:, ltm:, m0:m1]   # [p, lts, g, t, bc, ri]
                cP = Cv[:, :, :, 0, :, :]     # [p, lts, g, bc, ri]
                cT = Cv[:, :, :, 1, :, :]
                sqg = fin.tile([128, LT * 8 * 64], bft, tag="sqg", bufs=2)
                sq_v = sqg[:, :nlt * gsz * 64].rearrange(
                    "p (lts g t bc ri) -> p lts g t bc ri", lts=nlt, g=gsz, t=T, bc=BC)
                nc.vector.tensor_tensor(sq_v, Cv, Cv, mult)
                sq_r = sqg[:, :nlt * gsz * 64].rearrange(
                    "p (lts g t bc ri) -> p lts t bc g ri", lts=nlt, g=gsz, t=T, bc=BC)
                if init:
                    nc.vector.tensor_reduce(ar_pp[:, ltm:], sq_r, axis=AX.XY, op=add)
                else:
                    rtmp = fin.tile([128, 96], fp32, tag="rtmp", bufs=2)
                    rt = rtmp[:, :nlt * 32].rearrange(
                        "p (lts t bc) -> p lts t bc", lts=nlt, t=T)
                    nc.vector.tensor_reduce(rt, sq_r, axis=AX.XY, op=add)
                    nc.vector.tensor_tensor(
                        ar_pp[:, ltm:], ar_pp[:, ltm:], rt, add)
                crg = fin.tile([128, LT * 8 * 32], bft, tag="crg", bufs=2)
                cr_v = crg[:, :nlt * gsz * 32].rearrange(
                    "p (lts g bc ri) -> p lts g bc ri", lts=nlt, g=gsz, bc=BC)
                nc.gpsimd.tensor_tensor(cr_v, cP, cT, mult)
                cr_r = crg[:, :nlt * gsz * 32].rearrange(
                    "p (lts g bc ri) -> p lts bc g ri", lts=nlt, g=gsz, bc=BC)
                if init:
                    nc.vector.tensor_reduce(ar_cr[:, ltm:], cr_r, axis=AX.XY, op=add)
                else:
                    ctmp = fin.tile([128, 48], fp32, tag="ctmp", bufs=2)
                    ct = ctmp[:, :nlt * 16].rearrange("p (lts bc) -> p lts bc", lts=nlt)
                    nc.vector.tensor_reduce(ct, cr_r, axis=AX.XY, op=add)
                    nc.vector.tensor_tensor(
                        ar_cr[:, ltm:], ar_cr[:, ltm:], ct, add)
                cig = fin.tile([128, 2 * LT * 8 * 16], bft, tag="cig", bufs=2)
                ci_v = cig[:, :2 * nlt * gsz * 16].rearrange(
                    "p (s lts g bc) -> p s lts g bc", s=2, lts=nlt, g=gsz)
                nc.gpsimd.tensor_tensor(
                    ci_v[:, 0], cP[:, :, :, :, 0], cT[:, :, :, :, 1], mult)
                nc.gpsimd.tensor_tensor(
                    ci_v[:, 1], cP[:, :, :, :, 1], cT[:, :, :, :, 0], mult)
                ci_r = cig[:, :2 * nlt * gsz * 16].rearrange(
                    "p (s lts g bc) -> p s lts bc g", s=2, lts=nlt, g=gsz)
                itmp = fin.tile([128, 96], fp32, tag="itmp", bufs=2)
                it = itmp[:, :2 * nlt * 16].rearrange(
                    "p (s lts bc) -> p s lts bc", s=2, lts=nlt)
                nc.vector.tensor_reduce(it, ci_r, axis=AX.X, op=add)
                if init:
                    nc.vector.tensor_tensor(ar_ci[:, ltm:], it[:, 0], it[:, 1], sub)
                else:
                    nc.vector.tensor_tensor(
                        ar_ci[:, ltm:], ar_ci[:, ltm:], it[:, 0], add)
                    nc.vector.tensor_tensor(
                        ar_ci[:, ltm:], ar_ci[:, ltm:], it[:, 1], sub)

            # ---- single AllGather of the bf16 payload + local tree-sum ----
            arh = fin.tile([128, 192], bft)
            nc.vector.tensor_copy(arh[:], ar_sb[:])
            nc.gpsimd.dma_start(ar_in[:, :], arh[:])
            nc.gpsimd.collective_compute(
                "AllGather", bypass,
                replica_groups=[list(range(NCORES))],
                ins=[ar_in[:, :]],
                outs=[ar_out[:, :]],
            )
            gall = fin.tile([128, NCORES, 192], bft)
            nc.sync.dma_start(
                gall[:], ar_out[:].rearrange("(r p) c -> p r c", r=NCORES))
            g4 = fin.tile([128, 4, 192], fp32)
            nc.vector.tensor_tensor(g4[:], gall[:, 0:4, :], gall[:, 4:8, :], add)
            g2 = fin.tile([128, 2, 192], fp32)
            nc.vector.tensor_tensor(g2[:], g4[:, 0:2, :], g4[:, 2:4, :], add)
            gA = fin.tile([128, 192], fp32)
            nc.vector.tensor_tensor(gA[:], g2[:, 0, :], g2[:, 1, :], add)

            # ---- final loss math.  legw carries sqrt(2) (the reference's
            # p = 2s scale) times LSC for fp8, so the gathered sums are
            # SC x reference; EPS consts scale to match and wvec divides SC
            # back out. ----
            EPS1 = EPS * SC
            EPS2 = EPS * SC * SC
            ps16 = fps.tile([16, 1], fp32, tag="ps16", bufs=1)
            zb = fin.tile([128, 1], fp32)
            nc.vector.memset(zb[:], 0.0)
            e2b = fin.tile([128, 1], fp32)
            nc.vector.memset(e2b[:], EPS2)

            def emit_final(ppf, crf, cif, lt_lo, lt_hi, sfx):
                nl = lt_hi - lt_lo
                n16 = nl * 16
                ppb = fin.tile([128, nl * 32], fp32, name=f"ppb{sfx}")
                nc.vector.tensor_scalar(ppb[:], ppf, EPS1, None, add)
                ppt = ppb[:].rearrange("p (lt t bc) -> p lt t bc", lt=nl, t=T)
                p0 = ppt[:, :, 0, :]
                p1 = ppt[:, :, 1, :]
                sqp = fin.tile([128, nl * 32], fp32, name=f"sqp{sfx}")
                nc.scalar.activation(sqp[:], ppb[:], AF.Sqrt, bias=zb[:])
                sqv = sqp[:].rearrange("p (lt t bc) -> p lt t bc", lt=nl, t=T)
                d = fin.tile([128, n16], fp32, name=f"d{sfx}")
                nc.vector.tensor_tensor(
                    d[:].rearrange("p (lt bc) -> p lt bc", lt=nl),
                    sqv[:, :, 0, :], sqv[:, :, 1, :], sub)
                amp = fin.tile([128, n16], fp32, name=f"amp{sfx}")
                nc.vector.tensor_tensor(amp[:], d[:], d[:], mult)
                msr = fin.tile([128, n16], fp32, name=f"msr{sfx}")
                nc.vector.tensor_tensor(msr[:], crf, crf, mult)
                msi = fin.tile([128, n16], fp32, name=f"msi{sfx}")
                nc.vector.tensor_tensor(msi[:], cif, cif, mult)
                msum = fin.tile([128, n16], fp32, name=f"msum{sfx}")
                nc.vector.tensor_tensor(msum[:], msr[:], msi[:], add)
                mag = fin.tile([128, n16], fp32, name=f"mag{sfx}")
                nc.scalar.activation(mag[:], msum[:], AF.Sqrt, bias=zb[:])
                dprod = fin.tile([128, n16], fp32, name=f"dprod{sfx}")
                nc.vector.tensor_tensor(
                    dprod[:].rearrange("p (lt bc) -> p lt bc", lt=nl), p0, p1, mult)
                denom = fin.tile([128, n16], fp32, name=f"denom{sfx}")
                nc.scalar.activation(denom[:], dprod[:], AF.Sqrt, bias=e2b[:])
                dpe = fin.tile([128, n16], fp32, name=f"dpe{sfx}")
                nc.vector.tensor_scalar(dpe[:], denom[:], EPS1, None, add)
                rec = fin.tile([128, n16], fp32, name=f"rec{sfx}")
                nc.vector.reciprocal(rec[:], dpe[:])
                coh = fin.tile([128, n16], fp32, name=f"coh{sfx}")
                nc.vector.tensor_tensor(coh[:], mag[:], rec[:], mult)
                cohc = fin.tile([128, n16], fp32, name=f"cohc{sfx}")
                nc.vector.tensor_scalar(cohc[:], coh[:], 1.0, 0.0, amin, amax)
                mx = fin.tile([128, n16], fp32, name=f"mx{sfx}")
                nc.vector.tensor_tensor(
                    mx[:].rearrange("p (lt bc) -> p lt bc", lt=nl), p0, p1, amax)
                onemc = fin.tile([128, n16], fp32, name=f"onemc{sfx}")
                nc.vector.tensor_scalar(onemc[:], cohc[:], -1.0, 1.0, mult, add)
                dec = fin.tile([128, n16], fp32, name=f"dec{sfx}")
                nc.vector.scalar_tensor_tensor(dec[:], mx[:], 2.0, onemc[:], mult, mult)
                tot = fin.tile([128, n16], fp32, name=f"tot{sfx}")
                nc.vector.tensor_tensor(tot[:], dec[:], amp[:], add)
                totv = tot[:].rearrange("p (lt bc) -> p lt bc", lt=nl)
                for j, lt in enumerate(range(lt_lo, lt_hi)):
                    nc.tensor.matmul(ps16[:], totv[:, j, :], mask_sb[:, lt:lt + 1],
                                     start=(lt == 0), stop=(lt == LT - 1))

            emit_final(gA[:, 0:96].rearrange("p (lt t bc) -> p lt t bc", lt=LT, t=T),
                       gA[:, 96:144].rearrange("p (lt bc) -> p lt bc", lt=LT),
                       gA[:, 144:192].rearrange("p (lt bc) -> p lt bc", lt=LT),
                       0, LT, "a")

            pc = fin.tile([16, 1], fp32)
            nc.vector.tensor_tensor(pc[:], ps16[:], wvec_sb[:], mult)
            ps1 = fps.tile([1, 1], fp32, tag="ps1", bufs=1)
            nc.tensor.matmul(ps1[:], pc[:], ones_sb[:], start=True, stop=True)
            osb = fin.tile([1, 1], fp32)
            nc.any.tensor_copy(osb[:], ps1[:])
            nc.sync.dma_start(out_e[:, :], osb[:])

    nc.compile()
    return nc


def make_in_maps(prediction, target, weights, leg, w):
    if "tables" not in _CACHE:
        _CACHE["tables"] = _build_tables(leg, w, weights)
        _CACHE["w_id"] = np.asarray(weights, np.float32).copy()
    legw, dftc, wvec, ones16, lmask = _CACHE["tables"]
    if not np.array_equal(_CACHE["w_id"], np.asarray(weights, np.float32)):
        wvec = (np.tile(np.asarray(weights, np.float32), T) / (360.0 * 16.0 * SC)).reshape(16, 1)

    xTE, xTO = _pack_inputs(prediction, target)
    return [
        {
            "xT": xTE if cid % 2 == 0 else xTO,
            "legw": legw[cid],
            "dftT": dftc[cid],
            "wvec": wvec,
            "ones16": ones16,
            "lmask": lmask,
        }
        for cid in range(NCORES)
    ]


def kernel(prediction, target, weights, leg, w):
    from concourse.bass_utils import run_bass_kernel_spmd

    if "graph" not in _CACHE:
        _CACHE["graph"] = _build_graph()
    nc = _CACHE["graph"]

    in_maps = make_in_maps(prediction, target, weights, leg, w)
    res = run_bass_kernel_spmd(nc, in_maps, core_ids=list(range(NCORES)))
    out = np.asarray(res.results[0]["out"], np.float32).reshape(())
    return out


# revision 20
# speedup vs baseline: 1.3360x; 1.0295x over previous
"""Distributed Trainium2 Bass kernel for the spherical-harmonic AMSE loss.

Algorithm (8 NeuronCores, m-sharded; m = 8k + core_id interleave):
  host:    longitude fold — F_m = sum_{n<360} (x[n] + (-1)^m x[n+360]) w^{mn};
           every m on core cid has parity cid%2, so each core gets its own
           folded xT with per-parity sign.  xT is packed [120(p), 96(t,bc,jt),
           3(kt), 128(j)] fp8 so each DMA chunk is a linear per-partition run.
  stage 1: flipped DFT — xT tiles [120,128] are the PE *stationary*, the DFT
           twiddle block [120, kt, 92] bf16 is the moving operand.  Output
           F arrives already j-partitioned ([128 j', 92 m2] per (t,bc,jt)
           tile), so no DMA transpose is needed and the PE rolls straight
           into stage 2 (keeps the p-state ramp).
  stage 2: C[l', (m,t,bc,ri)] = legw.T @ FT   (PE, PSUM accum over j-tiles);
           legw streams fp8: k<16 on the gpsimd SW-DGE from t=0, the rest on
           the sync/scalar HW queues behind the xT chunks.
  stage 3: |C|^2 and conj(P)*T products + reductions over local m, chunked
           by m-group so it overlaps stage 2.  Products split across DVE and
           Pool; reductions on DVE; PSUM->SBUF copies on Scalar.
  Single AllGather of the merged bf16 partial payload [128, 192] (pp|cr|ci),
  local tree-sum, final loss math redundantly per core.
"""
import os
import numpy as np
import ml_dtypes

os.environ.setdefault("NEURON_RT_DBG_RDH_CC", "0")
os.environ.setdefault("TILE_SCHEDULER", "asap")

NLON = 720
NLONF = 360          # folded longitude
L = 361
EPS = 1e-7
NCORES = 8
MSLOT = 46           # m slots per core (m = 8k + core_id; zero-padded if > 360)
M2 = 2 * MSLOT       # 92 live re/im columns
JP = 384             # padded latitude rows per (t, bc)  (3 * 128)
T = 2
BC = 16
KT = 3               # folded: 360 = 3 * 120
KTW = 120
LP = 384             # padded l (3 * 128)
LT = 3
TILES = T * BC * 3   # 96 (t, bc, jt) stationary tiles
TPC = 8              # tiles per DMA chunk
NCH = TILES // TPC   # 12 chunks
TPB = 4              # tiles per PSUM bank in stage 1
NB = TILES // TPB    # 24 banks

bf16 = ml_dtypes.bfloat16
f8 = ml_dtypes.float8_e4m3
LSC = 256.0              # fp8 legw pre-scale (keeps values in normal range)
SC = LSC * LSC           # pp/cr/ci come out scaled by LSC^2; wvec unwinds it

_CACHE = {}


def _build_tables(leg, w, weights):
    legf = np.asarray(leg, np.float32)          # [L, M, J]
    wf = np.asarray(w, np.float32)              # [J]
    legT = legf.transpose(1, 2, 0) * wf[None, :, None]   # [M, J, L]
    legT[0] *= np.float32(2.0 ** -0.5)          # uniform p = 2*sum|C|^2
    legT *= np.float32(2.0 ** 0.5)              # bake the psd 2x
    legT *= np.float32(LSC)                     # fp8 normal-range pre-scale
    legp = np.zeros((MSLOT * NCORES, JP, LP), np.float32)
    legp[:L, :L, :L] = legT
    legp = legp.reshape(MSLOT, NCORES, JP, LP).transpose(1, 0, 2, 3)
    legw = np.ascontiguousarray(
        legp.reshape(NCORES, MSLOT, 3, 128, LP).transpose(0, 1, 3, 2, 4)
    ).astype(f8)                                # [8][46, 128(j'), 3(jt), 384(l)]

    n = np.arange(NLONF, dtype=np.float64)
    m_all = np.arange(MSLOT * NCORES, dtype=np.float64)
    ang = 2.0 * np.pi * np.outer(n, m_all) / NLON
    scale = 2.0 * np.pi / NLON
    dft = np.zeros((NLONF, MSLOT * NCORES, 2), np.float64)
    dft[:, :, 0] = np.cos(ang) * scale
    dft[:, :, 1] = -np.sin(ang) * scale
    dft[:, L:, :] = 0.0
    dft = dft.reshape(NLONF, MSLOT, NCORES, 2).transpose(2, 0, 1, 3)  # [8,360,46,2]
    dft = dft.reshape(NCORES, KT, KTW, M2).transpose(0, 2, 1, 3)      # [8,120,3,92]
    dftc = np.ascontiguousarray(dft).astype(bf16)

    wvec = (np.tile(np.asarray(weights, np.float32), T) / (360.0 * 16.0 * SC)).reshape(16, 1)
    ones16 = np.ones((16, 1), np.float32)
    lmask = np.zeros((128, LT), np.float32)
    for lt in range(LT):
        for p in range(128):
            if lt * 128 + p < L - 1:
                lmask[p, lt] = 1.0
    return legw, dftc, wvec, ones16, lmask


def _pack_inputs(prediction, target):
    x = np.zeros((T, BC, JP, NLON), np.float32)
    x[0, :, :L] = np.asarray(prediction, np.float32).reshape(BC, L, NLON)
    x[1, :, :L] = np.asarray(target, np.float32).reshape(BC, L, NLON)
    lo, hi = x[..., :NLONF], x[..., NLONF:]

    def pack(xf):
        # xf [T, BC, JP, 360] -> [120(p), (t bc jt), kt, 128(j)]
        a = xf.transpose(3, 0, 1, 2)                 # [360, T, BC, JP]
        a = a.reshape(KT, KTW, T, BC, 3, 128)        # [kt, p, t, bc, jt, jc]
        a = a.transpose(1, 2, 3, 4, 0, 5)            # [p, t, bc, jt, kt, jc]
        return np.ascontiguousarray(a.reshape(KTW, TILES, KT, 128)).astype(f8)

    return pack(lo + hi), pack(lo - hi)


def _build_graph():
    import concourse.bacc as bacc
    import concourse.mybir as mybir
    from concourse.tile import TileContext

    fp32 = mybir.dt.float32
    bft = mybir.dt.bfloat16
    f8t = mybir.dt.float8e4

    nc = bacc.Bacc(None, target_bir_lowering=False)

    xT_e = nc.declare_dram_parameter("xT", [KTW, TILES, KT, 128], f8t, isOutput=False)
    legw_e = nc.declare_dram_parameter("legw", [MSLOT, 128, 3, LP], f8t, isOutput=False)
    dft_e = nc.declare_dram_parameter("dftT", [KTW, KT, M2], bft, isOutput=False)
    wvec_e = nc.declare_dram_parameter("wvec", [16, 1], fp32, isOutput=False)
    ones_e = nc.declare_dram_parameter("ones16", [16, 1], fp32, isOutput=False)
    mask_e = nc.declare_dram_parameter("lmask", [128, LT], fp32, isOutput=False)
    out_e = nc.declare_dram_parameter("out", [1, 1], fp32, isOutput=True)

    ar_in = nc.dram_tensor("ar_in", [128, 192], bft)
    ar_out = nc.dram_tensor("ar_out", [NCORES * 128, 192], bft, addr_space="Shared")
    wu_in = nc.dram_tensor("wu_in", [128, 2], bft)
    wu_out = nc.dram_tensor("wu_out", [NCORES * 128, 2], bft, addr_space="Shared")

    add = mybir.AluOpType.add
    sub = mybir.AluOpType.subtract
    mult = mybir.AluOpType.mult
    amax = mybir.AluOpType.max
    amin = mybir.AluOpType.min
    bypass = mybir.AluOpType.bypass
    AF = mybir.ActivationFunctionType
    AX = mybir.AxisListType

    with TileContext(nc) as tc:
        with (
            tc.tile_pool(name="consts", bufs=1) as consts,
            tc.tile_pool(name="xp", bufs=NCH) as xp,
            tc.tile_pool(name="fps", bufs=4, space="PSUM") as fps,
            tc.tile_pool(name="big", bufs=1) as big,
            tc.tile_pool(name="lw", bufs=32) as lwp,
            tc.tile_pool(name="cps", bufs=3, space="PSUM") as cps,
            tc.tile_pool(name="fin", bufs=1) as fin,
        ):
            # ---- warm-up collective: absorbs the RDH first-collective
            # setup + barrier so the real AllGather runs warm ----
            wu_sb = consts.tile([128, 2], bft)
            nc.vector.memset(wu_sb[:], 0.0)
            nc.gpsimd.dma_start(wu_in[:, :], wu_sb[:])
            nc.gpsimd.collective_compute(
                "AllGather", mybir.AluOpType.bypass,
                replica_groups=[list(range(NCORES))],
                ins=[wu_in[:, :]],
                outs=[wu_out[:, :]],
            )

            dft_sb = consts.tile([KTW, KT, M2], bft)
            nc.sync.dma_start(dft_sb[:], dft_e[:])
            wvec_sb = consts.tile([16, 1], fp32)
            nc.sync.dma_start(wvec_sb[:], wvec_e[:])
            ones_sb = consts.tile([16, 1], fp32)
            nc.sync.dma_start(ones_sb[:], ones_e[:])
            mask_sb = consts.tile([128, LT], fp32)
            nc.sync.dma_start(mask_sb[:], mask_e[:])

            # ---- legw streaming: k<16 rides the gpsimd SW-DGE from t=0 so
            # that queue is drained well before the collective triggers; the
            # rest ride the sync/scalar HW queues behind the xT chunks ----
            MGROUPS = [(0, 8), (8, 16), (16, 24), (24, 32), (32, 38), (38, 43), (43, 46)]
            LTMIN = [0, 0, 1, 1, 2, 2, 2]
            lw_tiles = {}

            def load_lw(k, eng):
                lt0 = min(2, k // 16)   # l-tiles below the diagonal are zero
                lw = lwp.tile([128, 3, LP], f8t, name="lw")
                eng.dma_start(lw[:, :, lt0 * 128:LP],
                              legw_e[k][:, :, lt0 * 128:LP])
                lw_tiles[k] = lw

            for k in range(20):
                load_lw(k, nc.gpsimd)

            # ---- stage 1: flipped DFT ----
            FT_sb = big.tile([128, TILES * M2], bft)     # [j', (t bc jt m2)]
            xch = []
            for g in range(NCH):
                xt = xp.tile([KTW, TPC, KT, 128], f8t, name="xch")
                eng = nc.sync if g % 2 == 0 else nc.scalar
                eng.dma_start(xt[:], xT_e[:, g * TPC:(g + 1) * TPC, :, :])
                xch.append(xt)
            for k in range(20, MSLOT):
                load_lw(k, nc.sync if k % 2 == 0 else nc.scalar)
            for b in range(NB):
                ps = fps.tile([128, 512], fp32, tag="s1", bufs=3)
                for i in range(TPB):
                    tl = b * TPB + i
                    xt = xch[tl // TPC]
                    for kt in range(KT):
                        nc.tensor.matmul(
                            ps[:, i * M2:(i + 1) * M2],
                            xt[:, tl % TPC, kt, :],
                            dft_sb[:, kt, :],
                            start=(kt == 0), stop=(kt == KT - 1),
                        )
                dst = FT_sb[:, b * TPB * M2:(b + 1) * TPB * M2]
                if b % 2 == 0:
                    nc.scalar.activation(dst, ps[:, :TPB * M2], AF.Copy)
                else:
                    nc.vector.tensor_copy(dst, ps[:, :TPB * M2])
            FT_v = FT_sb[:].rearrange(
                "p (t bc jt m) -> p t bc jt m", t=T, bc=BC, jt=3, m=M2
            )

            # ---- stage 2 + stage 3 per m-group ----
            # Call holds all C tiles [p, (lt, k, t, bc, ri)] at absolute m
            # slots so stage-3 ops can span a group's full lt range.
            Call = big.tile([128, LT * MSLOT * 64], bft)
            Cv_all = Call[:].rearrange(
                "p (lt k t bc ri) -> p lt k t bc ri", lt=LT, k=MSLOT, t=T, bc=BC)
            # merged all-reduce payload: [0:96] pp (lt,t,bc) | [96:144] cr
            # (lt,bc) | [144:192] ci (lt,bc)
            ar_sb = fin.tile([128, 192], fp32)
            ar_pp = ar_sb[:, 0:96].rearrange("p (lt t bc) -> p lt t bc", lt=LT, t=T)
            ar_cr = ar_sb[:, 96:144].rearrange("p (lt bc) -> p lt bc", lt=LT)
            ar_ci = ar_sb[:, 144:192].rearrange("p (lt bc) -> p lt bc", lt=LT)
            for gi, (m0, m1) in enumerate(MGROUPS):
                gsz = m1 - m0
                ltm = LTMIN[gi]
                nlt = LT - ltm
                init = gi == 0
                for lt in range(ltm, LT):
                    ps = cps.tile([128, 512], fp32)
                    for mi in range(gsz):
                        k = m0 + mi
                        rhs = FT_v[:, :, :, :, 2 * k:2 * k + 2]
                        for jt in range(3):
                            nc.tensor.matmul(
                                ps[:, mi * 64:(mi + 1) * 64],
                                lw_tiles[k][:, jt, lt * 128:(lt + 1) * 128],
                                rhs[:, :, :, jt, :],
                                start=(jt == 0), stop=(jt == 2),
                            )
                    nc.scalar.activation(
                        Call[:, (lt * MSLOT + m0) * 64:(lt * MSLOT + m1) * 64],
                        ps[:, 0:gsz * 64], AF.Copy)
                # ---- stage 3 for this m-group (overlaps next group's matmuls).
                # products: sq on DVE, cr/ci on Pool; reductions on DVE ----
                Cv = Cv_all[:, ltm:, m0:m1]   # [p, lts, g, t, bc, ri]
                cP = Cv[:, :, :, 0, :, :]     # [p, lts, g, bc, ri]
                cT = Cv[:, :, :, 1, :, :]
                sqg = fin.tile([128, LT * 8 * 64], bft, tag="sqg", bufs=2)
                sq_v = sqg[:, :nlt * gsz * 64].rearrange(
                    "p (lts g t bc ri) -> p lts g t bc ri", lts=nlt, g=gsz, t=T, bc=BC)
                nc.vector.tensor_tensor(sq_v, Cv, Cv, mult)
                sq_r = sqg[:, :nlt * gsz * 64].rearrange(
                    "p (lts g t bc ri) -> p lts t bc g ri", lts=nlt, g=gsz, t=T, bc=BC)
                if init:
                    nc.vector.tensor_reduce(ar_pp[:, ltm:], sq_r, axis=AX.XY, op=add)
                else:
                    rtmp = fin.tile([128, 96], fp32, tag="rtmp", bufs=2)
                    rt = rtmp[:, :nlt * 32].rearrange(
                        "p (lts t bc) -> p lts t bc", lts=nlt, t=T)
                    nc.vector.tensor_reduce(rt, sq_r, axis=AX.XY, op=add)
                    nc.vector.tensor_tensor(
                        ar_pp[:, ltm:], ar_pp[:, ltm:], rt, add)
                crg = fin.tile([128, LT * 8 * 32], bft, tag="crg", bufs=2)
                cr_v = crg[:, :nlt * gsz * 32].rearrange(
                    "p (lts g bc ri) -> p lts g bc ri", lts=nlt, g=gsz, bc=BC)
                nc.gpsimd.tensor_tensor(cr_v, cP, cT, mult)
                cr_r = crg[:, :nlt * gsz * 32].rearrange(
                    "p (lts g bc ri) -> p lts bc g ri", lts=nlt, g=gsz, bc=BC)
                if init:
                    nc.vector.tensor_reduce(ar_cr[:, ltm:], cr_r, axis=AX.XY, op=add)
                else:
                    ctmp = fin.tile([128, 48], fp32, tag="ctmp", bufs=2)
                    ct = ctmp[:, :nlt * 16].rearrange("p (lts bc) -> p lts bc", lts=nlt)
                    nc.vector.tensor_reduce(ct, cr_r, axis=AX.XY, op=add)
                    nc.vector.tensor_tensor(
                        ar_cr[:, ltm:], ar_cr[:, ltm:], ct, add)
                cig = fin.tile([128, 2 * LT * 8 * 16], bft, tag="cig", bufs=2)
                ci_v = cig[:, :2 * nlt * gsz * 16].rearrange(
                    "p (s lts g bc) -> p s lts g bc", s=2, lts=nlt, g=gsz)
                nc.gpsimd.tensor_tensor(
                    ci_v[:, 0], cP[:, :, :, :, 0], cT[:, :, :, :, 1], mult)
                nc.gpsimd.tensor_tensor(
                    ci_v[:, 1], cP[:, :, :, :, 1], cT[:, :, :, :, 0], mult)
                ci_r = cig[:, :2 * nlt * gsz * 16].rearrange(
                    "p (s lts g bc) -> p s lts bc g", s=2, lts=nlt, g=gsz)
                itmp = fin.tile([128, 96], fp32, tag="itmp", bufs=2)
                it = itmp[:, :2 * nlt * 16].rearrange(
                    "p (s lts bc) -> p s lts bc", s=2, lts=nlt)
                nc.vector.tensor_reduce(it, ci_r, axis=AX.X, op=add)
                if init:
                    nc.vector.tensor_tensor(ar_ci[:, ltm:], it[:, 0], it[:, 1], sub)
                else:
                    nc.vector.tensor_tensor(
                        ar_ci[:, ltm:], ar_ci[:, ltm:], it[:, 0], add)
                    nc.vector.tensor_tensor(
                        ar_ci[:, ltm:], ar_ci[:, ltm:], it[:, 1], sub)

            # ---- single AllGather of the bf16 payload + local tree-sum ----
            arh = fin.tile([128, 192], bft)
            nc.vector.tensor_copy(arh[:], ar_sb[:])
            nc.gpsimd.dma_start(ar_in[:, :], arh[:])
            nc.gpsimd.collective_compute(
                "AllGather", bypass,
                replica_groups=[list(range(NCORES))],
                ins=[ar_in[:, :]],
                outs=[ar_out[:, :]],
            )
            gall = fin.tile([128, NCORES, 192], bft)
            nc.sync.dma_start(
                gall[:], ar_out[:].rearrange("(r p) c -> p r c", r=NCORES))
            g4 = fin.tile([128, 4, 192], fp32)
            nc.vector.tensor_tensor(g4[:], gall[:, 0:4, :], gall[:, 4:8, :], add)
            g2 = fin.tile([128, 2, 192], fp32)
            nc.vector.tensor_tensor(g2[:], g4[:, 0:2, :], g4[:, 2:4, :], add)
            gA = fin.tile([128, 192], fp32)
            nc.vector.tensor_tensor(gA[:], g2[:, 0, :], g2[:, 1, :], add)

            # ---- final loss math.  legw carries sqrt(2) (the reference's
            # p = 2s scale) times LSC for fp8, so the gathered sums are
            # SC x reference; EPS consts scale to match and wvec divides SC
            # back out. ----
            EPS1 = EPS * SC
            EPS2 = EPS * SC * SC
            ps16 = fps.tile([16, 1], fp32, tag="ps16", bufs=1)
            zb = fin.tile([128, 1], fp32)
            nc.vector.memset(zb[:], 0.0)
            e2b = fin.tile([128, 1], fp32)
            nc.vector.memset(e2b[:], EPS2)

            def emit_final(ppf, crf, cif, lt_lo, lt_hi, sfx):
                nl = lt_hi - lt_lo
                n16 = nl * 16
                ppb = fin.tile([128, nl * 32], fp32, name=f"ppb{sfx}")
                nc.vector.tensor_scalar(ppb[:], ppf, EPS1, None, add)
                ppt = ppb[:].rearrange("p (lt t bc) -> p lt t bc", lt=nl, t=T)
                p0 = ppt[:, :, 0, :]
                p1 = ppt[:, :, 1, :]
                sqp = fin.tile([128, nl * 32], fp32, name=f"sqp{sfx}")
                nc.scalar.activation(sqp[:], ppb[:], AF.Sqrt, bias=zb[:])
                sqv = sqp[:].rearrange("p (lt t bc) -> p lt t bc", lt=nl, t=T)
                d = fin.tile([128, n16], fp32, name=f"d{sfx}")
                nc.vector.tensor_tensor(
                    d[:].rearrange("p (lt bc) -> p lt bc", lt=nl),
                    sqv[:, :, 0, :], sqv[:, :, 1, :], sub)
                amp = fin.tile([128, n16], fp32, name=f"amp{sfx}")
                nc.vector.tensor_tensor(amp[:], d[:], d[:], mult)
                msr = fin.tile([128, n16], fp32, name=f"msr{sfx}")
                nc.vector.tensor_tensor(msr[:], crf, crf, mult)
                msi = fin.tile([128, n16], fp32, name=f"msi{sfx}")
                nc.vector.tensor_tensor(msi[:], cif, cif, mult)
                msum = fin.tile([128, n16], fp32, name=f"msum{sfx}")
                nc.vector.tensor_tensor(msum[:], msr[:], msi[:], add)
                mag = fin.tile([128, n16], fp32, name=f"mag{sfx}")
                nc.scalar.activation(mag[:], msum[:], AF.Sqrt, bias=zb[:])
                dprod = fin.tile([128, n16], fp32, name=f"dprod{sfx}")
                nc.vector.tensor_tensor(
                    dprod[:].rearrange("p (lt bc) -> p lt bc", lt=nl), p0, p1, mult)
                denom = fin.tile([128, n16], fp32, name=f"denom{sfx}")
                nc.scalar.activation(denom[:], dprod[:], AF.Sqrt, bias=e2b[:])
                dpe = fin.tile([128, n16], fp32, name=f"dpe{sfx}")
                nc.vector.tensor_scalar(dpe[:], denom[:], EPS1, None, add)
                rec = fin.tile([128, n16], fp32, name=f"rec{sfx}")
                nc.vector.reciprocal(rec[:], dpe[:])
                coh = fin.tile([128, n16], fp32, name=f"coh{sfx}")
                nc.vector.tensor_tensor(coh[:], mag[:], rec[:], mult)
                cohc = fin.tile([128, n16], fp32, name=f"cohc{sfx}")
                nc.vector.tensor_scalar(cohc[:], coh[:], 1.0, 0.0, amin, amax)
                mx = fin.tile([128, n16], fp32, name=f"mx{sfx}")
                nc.vector.tensor_tensor(
                    mx[:].rearrange("p (lt bc) -> p lt bc", lt=nl), p0, p1, amax)
                onemc = fin.tile([128, n16], fp32, name=f"onemc{sfx}")
                nc.vector.tensor_scalar(onemc[:], cohc[:], -1.0, 1.0, mult, add)
                dec = fin.tile([128, n16], fp32, name=f"dec{sfx}")
                nc.vector.scalar_tensor_tensor(dec[:], mx[:], 2.0, onemc[:], mult, mult)
                tot = fin.tile([128, n16], fp32, name=f"tot{sfx}")
                nc.vector.tensor_tensor(tot[:], dec[:], amp[:], add)
                totv = tot[:].rearrange("p (lt bc) -> p lt bc", lt=nl)
                for j, lt in enumerate(range(lt_lo, lt_hi)):
                    nc.tensor.matmul(ps16[:], totv[:, j, :], mask_sb[:, lt:lt + 1],
                                     start=(lt == 0), stop=(lt == LT - 1))

            emit_final(gA[:, 0:96].rearrange("p (lt t bc) -> p lt t bc", lt=LT, t=T),
                       gA[:, 96:144].rearrange("p (lt bc) -> p lt bc", lt=LT),
                       gA[:, 144:192].rearrange("p (lt bc) -> p lt bc", lt=LT),
                       0, LT, "a")

            pc = fin.tile([16, 1], fp32)
            nc.vector.tensor_tensor(pc[:], ps16[:], wvec_sb[:], mult)
            ps1 = fps.tile([1, 1], fp32, tag="ps1", bufs=1)
            nc.tensor.matmul(ps1[:], pc[:], ones_sb[:], start=True, stop=True)
            osb = fin.tile([1, 1], fp32)
            nc.any.tensor_copy(osb[:], ps1[:])
            nc.sync.dma_start(out_e[:, :], osb[:])

    nc.compile()
    return nc


def make_in_maps(prediction, target, weights, leg, w):
    if "tables" not in _CACHE:
        _CACHE["tables"] = _build_tables(leg, w, weights)
        _CACHE["w_id"] = np.asarray(weights, np.float32).copy()
    legw, dftc, wvec, ones16, lmask = _CACHE["tables"]
    if not np.array_equal(_CACHE["w_id"], np.asarray(weights, np.float32)):
        wvec = (np.tile(np.asarray(weights, np.float32), T) / (360.0 * 16.0 * SC)).reshape(16, 1)

    xTE, xTO = _pack_inputs(prediction, target)
    return [
        {
            "xT": xTE if cid % 2 == 0 else xTO,
            "legw": legw[cid],
            "dftT": dftc[cid],
            "wvec": wvec,
            "ones16": ones16,
            "lmask": lmask,
        }
        for cid in range(NCORES)
    ]


def kernel(prediction, target, weights, leg, w):
    from concourse.bass_utils import run_bass_kernel_spmd

    if "graph" not in _CACHE:
        _CACHE["graph"] = _build_graph()
    nc = _CACHE["graph"]

    in_maps = make_in_maps(prediction, target, weights, leg, w)
    res = run_bass_kernel_spmd(nc, in_maps, core_ids=list(range(NCORES)))
    out = np.asarray(res.results[0]["out"], np.float32).reshape(())
    return out
